# revision 4
# baseline (speedup 1.0000x reference)
"""Trainium2 Bass kernel for varlen GQA cross-attention (4 seqs x 2048 q, 512 kv).

Strategy: data-parallel over query rows. Each of the 8 cores owns 1024 query
rows (half of one sequence) and the full 512-slot KV of that sequence.
No collectives needed.

Per-core dataflow (all layouts chosen so no on-device transposes are needed):
  xT [4096,1024] (host pre-transposed)  -> Q^T = Wq.T-chunks x xT   [hd, q]
  RoPE on Q^T / K^T via pair-swap permutation matmul + cos/sin tables
  S^T[k,q] = (K^T chunk).T @ Q^T        (contraction over head_dim)
  expS = exp(S^T * scale)  (ScalarE, PSUM->SBUF)
  denom[1,q] = ones.T @ expS            (partition-dim reduction by matmul)
  O^T[hd,q] = V-chunk.T @ expS          (PSUM accum over k chunks)
  O^T *= broadcast(1/denom)             (broadcast via K=1 ones matmul)
  Y[q,n] = O^T-chunks.T @ Wo-chunks     (accumulate over all 32 heads)

Matmuls run as float32r (1 cycle/row on the PE at N=512); the final Wo stage
runs in bf16 (halves Wo DMA and the attention-output SBUF footprint).
"""

import sys

if "/opt/trn_rl_repo" not in sys.path:
    sys.path.insert(0, "/opt/trn_rl_repo")

import numpy as np
import ml_dtypes
from contextlib import ExitStack

import concourse.bass as bass
import concourse.tile as tile
import concourse.mybir as mybir
from concourse import bacc
from concourse.bass_utils import run_bass_kernel_spmd

# Problem constants (hardcoded per harness contract)
DIM = 4096
N_HEADS = 32
HEAD_DIM = 128
N_KV_HEADS = 8
REPEATS = N_HEADS // N_KV_HEADS
SCALE = HEAD_DIM ** -0.5
ROPE_THETA = 10000.0
NUM_SEQS = 4
Q_LEN = 2048
KV_LEN = 512
N_CORES = 8
RQ = (NUM_SEQS * Q_LEN) // N_CORES   # 1024 query rows per core
QP = 512                              # q rows per pass
N_PASS = RQ // QP                     # 2
P = 128

f32 = mybir.dt.float32
f32r = mybir.dt.float32r
bf16 = mybir.dt.bfloat16
Copy = mybir.ActivationFunctionType.Copy
Exp = mybir.ActivationFunctionType.Exp

_BUILT = None


def _build():
    """Build + compile the per-core Bass program (same NEFF on all 8 cores)."""
    global _BUILT
    if _BUILT is not None:
        return _BUILT

    nc = bacc.Bacc("TRN2", target_bir_lowering=False, debug=False,
                   num_devices=N_CORES)
    xt = nc.dram_tensor("xt", [DIM, RQ], f32r, kind="ExternalInput").ap()
    wq = nc.dram_tensor("wq", [N_HEADS, P, DIM // P, P], f32r,
                        kind="ExternalInput").ap()
    wo = nc.dram_tensor("wo", [DIM, DIM], bf16, kind="ExternalInput").ap()
    ktd = nc.dram_tensor("kt", [N_KV_HEADS * P, KV_LEN], f32r,
                         kind="ExternalInput").ap()
    vd = nc.dram_tensor("v", [KV_LEN, N_KV_HEADS * P], f32r,
                        kind="ExternalInput").ap()
    cqd = nc.dram_tensor("cq", [N_PASS, P, QP], f32, kind="ExternalInput").ap()
    sqd = nc.dram_tensor("sq", [N_PASS, P, QP], f32, kind="ExternalInput").ap()
    ckd = nc.dram_tensor("ck", [P, KV_LEN], f32, kind="ExternalInput").ap()
    skd = nc.dram_tensor("sk", [P, KV_LEN], f32, kind="ExternalInput").ap()
    pmd = nc.dram_tensor("pm", [P, P], f32r, kind="ExternalInput").ap()
    ond = nc.dram_tensor("on", [P, P], f32r, kind="ExternalInput").ap()
    out = nc.dram_tensor("out", [RQ, DIM], f32, kind="ExternalOutput").ap()

    KC = DIM // P  # 32 k-chunks of the model dim

    with tile.TileContext(nc) as tc:
        with ExitStack() as ctx:
            singles = ctx.enter_context(tc.tile_pool(name="singles", bufs=1))
            big = ctx.enter_context(tc.tile_pool(name="big", bufs=1))
            wqp = ctx.enter_context(tc.tile_pool(name="wqp", bufs=3))
            wop = ctx.enter_context(tc.tile_pool(name="wop", bufs=3))
            wk = ctx.enter_context(tc.tile_pool(name="wk", bufs=2))
            esp = ctx.enter_context(tc.tile_pool(name="esp", bufs=6))
            psA = ctx.enter_context(tc.tile_pool(name="psA", bufs=4, space="PSUM"))
            psB = ctx.enter_context(tc.tile_pool(name="psB", bufs=2, space="PSUM"))
            psC = ctx.enter_context(tc.tile_pool(name="psC", bufs=2, space="PSUM"))

            # Resident tensors
            kt_sb = singles.tile([P, N_KV_HEADS, KV_LEN], f32r)
            v_sb = singles.tile([P, KV_LEN // P, N_KV_HEADS, P], f32r)
            pm_sb = singles.tile([P, P], f32r)
            on_sb = singles.tile([P, P], f32r)
            ck_sb = singles.tile([P, KV_LEN], f32)
            sk_sb = singles.tile([P, KV_LEN], f32)
            nc.sync.dma_start(kt_sb, ktd.rearrange("(g d) k -> d g k", d=P))
            nc.sync.dma_start(
                v_sb, vd.rearrange("(kc kin) (g hd) -> kin kc g hd", kin=P, hd=P))
            nc.sync.dma_start(pm_sb, pmd)
            nc.sync.dma_start(on_sb, ond)
            nc.sync.dma_start(ck_sb, ckd)
            nc.sync.dma_start(sk_sb, skd)

            # RoPE on K^T, in place: k = k*C + swap(k)*S
            for g in range(N_KV_HEADS):
                ksw = psB.tile([P, KV_LEN], f32, tag="st")
                nc.tensor.matmul(ksw, pm_sb,
                                 kt_sb[:, g],
                                 start=True, stop=True)
                kt1 = wk.tile([P, KV_LEN], f32, tag="vtmp")
                nc.vector.tensor_mul(kt1, ksw, sk_sb)
                nc.vector.tensor_mul(kt_sb[:, g], kt_sb[:, g], ck_sb)
                nc.vector.tensor_add(kt_sb[:, g], kt_sb[:, g], kt1)

            for ps in range(N_PASS):
                xt_sb = big.tile([P, KC, QP], f32r, tag="xt")
                nc.sync.dma_start(
                    xt_sb,
                    xt[:, ps * QP:(ps + 1) * QP].rearrange(
                        "(kc kin) q -> kin kc q", kin=P))
                ot_all = big.tile([P, N_HEADS, QP], bf16, tag="ot")
                cq_sb = big.tile([P, QP], f32, tag="cq")
                sq_sb = big.tile([P, QP], f32, tag="sq")
                nc.sync.dma_start(cq_sb, cqd[ps])
                nc.sync.dma_start(sq_sb, sqd[ps])

                for h in range(N_HEADS):
                    g = h // REPEATS
                    # ---- Phase 1: Q^T for head h over this q-pass ----
                    qtp = psA.tile([P, QP], f32, tag="mmout")
                    for kg in range(4):
                        wq_t = wqp.tile([P, KC // 4, P], f32r, tag="wq")
                        nc.sync.dma_start(wq_t, wq[h, :, kg * 8:(kg + 1) * 8, :])
                        for k2 in range(8):
                            k = kg * 8 + k2
                            nc.tensor.matmul(qtp,
                                             wq_t[:, k2],
                                             xt_sb[:, k],
                                             start=(k == 0), stop=(k == KC - 1))
                    # RoPE (transposed layout): qr = qt*C + perm(qt)*S
                    qt_raw = wk.tile([P, QP], f32r, tag="qt_raw")
                    nc.scalar.activation(qt_raw, qtp, Copy)
                    qsw = psB.tile([P, QP], f32, tag="st")
                    nc.tensor.matmul(qsw, pm_sb,
                                     qt_raw, start=True, stop=True)
                    t1 = wk.tile([P, QP], f32, tag="vtmp")
                    nc.vector.tensor_mul(t1, qsw, sq_sb)
                    qt_rope = wk.tile([P, QP], f32r, tag="qt_rope")
                    nc.vector.tensor_mul(qt_rope, qt_raw, cq_sb)
                    nc.vector.tensor_add(qt_rope, qt_rope, t1)

                    # ---- Phase 2: attention for head h ----
                    dnp = psC.tile([1, QP], f32, tag="aux")
                    otp = psA.tile([P, QP], f32, tag="mmout")
                    for kt_i in range(KV_LEN // P):
                        stp = psB.tile([P, QP], f32, tag="st")
                        nc.tensor.matmul(
                            stp,
                            kt_sb[:, g, kt_i * P:(kt_i + 1) * P],
                            qt_rope, start=True, stop=True)
                        es = esp.tile([P, QP], f32r, tag="es")
                        nc.scalar.activation(es, stp, Exp, scale=SCALE)
                        nc.tensor.matmul(dnp, on_sb[:, 0:1],
                                         es,
                                         start=(kt_i == 0),
                                         stop=(kt_i == KV_LEN // P - 1))
                        nc.tensor.matmul(otp,
                                         v_sb[:, kt_i, g],
                                         es,
                                         start=(kt_i == 0),
                                         stop=(kt_i == KV_LEN // P - 1))
                    rc = wk.tile([1, QP], f32r, tag="rc")
                    with nc.allow_low_precision(
                            reason="f32r == f32 bits; PE just streams it"):
                        nc.vector.reciprocal(rc, dnp)
                    rbp = psC.tile([P, QP], f32, tag="aux")
                    nc.tensor.matmul(rbp, on_sb[0:1, :],
                                     rc, start=True, stop=True)
                    rb = wk.tile([P, QP], f32, tag="vtmp")
                    nc.scalar.activation(rb, rbp, Copy)
                    nc.vector.tensor_mul(ot_all[:, h], otp, rb)

                # ---- Phase 3: Y = O @ Wo for this q-pass ----
                for n in range(DIM // 512):
                    yps = [psA.tile([P, 512], f32, tag="mmout",
                                    name=f"y_{ps}_{n}_{m}")
                           for m in range(QP // P)]
                    for h in range(N_HEADS):
                        wo_t = wop.tile([P, 512], bf16, tag="wo")
                        nc.sync.dma_start(
                            wo_t, wo[h * P:(h + 1) * P, n * 512:(n + 1) * 512])
                        for m in range(QP // P):
                            nc.tensor.matmul(yps[m],
                                             ot_all[:, h, m * P:(m + 1) * P],
                                             wo_t,
                                             start=(h == 0),
                                             stop=(h == N_HEADS - 1))
                    for m in range(QP // P):
                        ysb = wk.tile([P, 512], f32, tag="y")
                        nc.vector.tensor_copy(ysb, yps[m])
                        r0 = ps * QP + m * P
                        nc.sync.dma_start(
                            out[r0:r0 + P, n * 512:(n + 1) * 512], ysb)

    nc.compile()
    _BUILT = nc
    return nc


def _host_prep(x, xk, xv, Wq, Wo):
    """Build the per-core input maps (shard + layout only; no NN math)."""
    x = np.asarray(x, dtype=np.float32)
    xk = np.asarray(xk, dtype=np.float32)
    xv = np.asarray(xv, dtype=np.float32)
    Wq = np.asarray(Wq, dtype=np.float32)
    Wo = np.asarray(Wo, dtype=np.float32)

    # Shared (same on all cores)
    wq_blk = np.ascontiguousarray(
        Wq.reshape(DIM // P, P, N_HEADS, P).transpose(2, 1, 0, 3))
    wo_bf = Wo.astype(ml_dtypes.bfloat16)
    pm = np.zeros((P, P), np.float32)
    idx = np.arange(0, P, 2)
    pm[idx + 1, idx] = 1.0
    pm[idx, idx + 1] = 1.0
    ones = np.ones((P, P), np.float32)

    inv = ROPE_THETA ** (-np.arange(0, HEAD_DIM, 2, dtype=np.float32) / HEAD_DIM)

    def tables(pos):
        # C[d, t] = cos(pos[t] * invf[d//2]); S[d, t] = -/+ sin(...)
        ang = pos[None, :].astype(np.float32) * inv[:, None]  # [64, T]
        c = np.cos(ang)
        s = np.sin(ang)
        C = np.repeat(c, 2, axis=0)
        S = np.repeat(s, 2, axis=0)
        S[0::2] *= -1.0
        return np.ascontiguousarray(C), np.ascontiguousarray(S)

    ck, sk = tables(np.arange(KV_LEN))

    in_maps = []
    for c in range(N_CORES):
        r0 = c * RQ
        b = r0 // Q_LEN
        qoff = r0 % Q_LEN
        xt_c = np.ascontiguousarray(x[r0:r0 + RQ].T)
        kt_c = np.ascontiguousarray(xk[b * KV_LEN:(b + 1) * KV_LEN].T)
        v_c = np.ascontiguousarray(xv[b * KV_LEN:(b + 1) * KV_LEN])
        cq = np.empty((N_PASS, P, QP), np.float32)
        sq = np.empty((N_PASS, P, QP), np.float32)
        for p_i in range(N_PASS):
            Cq, Sq = tables(qoff + p_i * QP + np.arange(QP))
            cq[p_i] = Cq
            sq[p_i] = Sq
        in_maps.append({
            "xt": xt_c, "wq": wq_blk, "wo": wo_bf, "kt": kt_c, "v": v_c,
            "cq": cq, "sq": sq, "ck": ck, "sk": sk, "pm": pm, "on": ones,
        })
    return in_maps


def run_sharded(inputs, trace=False, trace_kwargs=None):
    """Build/compile (cached), run on cores 0-7, return (full_out, results)."""
    nc = _build()
    in_maps = _host_prep(inputs["x"], inputs["xk"], inputs["xv"],
                         inputs["Wq"], inputs["Wo"])
    kw = {}
    if trace:
        kw["trace"] = True
        if trace_kwargs:
            kw["trace_kwargs"] = trace_kwargs
    res = run_bass_kernel_spmd(nc, in_maps, core_ids=list(range(N_CORES)), **kw)
    full = np.concatenate([res.results[c]["out"] for c in range(N_CORES)],
                          axis=0)
    return full, res


def kernel(**inputs):
    ns = inputs.get("num_seqs", NUM_SEQS)
    assert int(ns) == NUM_SEQS, f"kernel hardcoded for num_seqs={NUM_SEQS}"
    full, _ = run_sharded(inputs, trace=False)
    return full


# revision 7
# speedup vs baseline: 1.2405x; 1.2405x over previous
"""Trainium2 Bass kernel for varlen GQA cross-attention (4 seqs x 2048 q, 512 kv).

Strategy: data-parallel over query rows. Each of the 8 cores owns 1024 query
rows (half of one sequence) and the full 512-slot KV of that sequence.
No collectives needed.

Per-core dataflow (layouts chosen so no on-device transposes are needed):
  xT [4096,1024] (host pre-transposed)  -> Q^T = Wq.T-chunks x xT   [hd, q]
  RoPE on Q^T / K^T via pair-swap permutation matmul + cos/sin tables
  S^T[k,q] = (K^T chunk).T @ Q^T        (contraction over head_dim)
  expS = exp(S^T * scale)  (ScalarE, PSUM->SBUF)
  denom[1,q] = ones.T @ expS            (partition-dim reduction by matmul)
  O^T[hd,q] = V-chunk.T @ expS          (PSUM accum over k chunks)
  O^T *= broadcast(1/denom)             (broadcast via K=1 ones matmul)
  Y[q,n] = O^T-chunks.T @ Wo-chunks     (accumulate over all 32 heads)

All matmul operands are fp16 (1 cycle/row on the PE, fast weight load that
overlaps with matmuls); every accumulation is fp32 in PSUM, and softmax
intermediates stay fp32 on the vector/scalar engines.

The per-head attention chain (PE -> ACT -> PE -> DVE -> PE) is software
pipelined one head deep so the PE always has the next head's 32 independent
Q-projection matmuls to execute while a head's cross-engine chain resolves.
"""

import sys

if "/opt/trn_rl_repo" not in sys.path:
    sys.path.insert(0, "/opt/trn_rl_repo")

import numpy as np
import ml_dtypes
from contextlib import ExitStack

import concourse.bass as bass
import concourse.tile as tile
import concourse.mybir as mybir
from concourse import bacc
from concourse.bass_utils import run_bass_kernel_spmd

# Problem constants (hardcoded per harness contract)
DIM = 4096
N_HEADS = 32
HEAD_DIM = 128
N_KV_HEADS = 8
REPEATS = N_HEADS // N_KV_HEADS
SCALE = HEAD_DIM ** -0.5
ROPE_THETA = 10000.0
NUM_SEQS = 4
Q_LEN = 2048
KV_LEN = 512
N_CORES = 8
RQ = (NUM_SEQS * Q_LEN) // N_CORES   # 1024 query rows per core
QP = 512                              # q rows per pass
N_PASS = RQ // QP                     # 2
P = 128
KC = DIM // P                         # 32 contraction chunks
NKT = KV_LEN // P                     # 4 kv chunks

f32 = mybir.dt.float32
f16 = mybir.dt.float16
Copy = mybir.ActivationFunctionType.Copy
Exp = mybir.ActivationFunctionType.Exp

_BUILT = None


def _build():
    """Build + compile the per-core Bass program (same NEFF on all 8 cores)."""
    global _BUILT
    if _BUILT is not None:
        return _BUILT

    nc = bacc.Bacc("TRN2", target_bir_lowering=False, debug=False,
                   num_devices=N_CORES)
    xt = nc.dram_tensor("xt", [DIM, RQ], f16, kind="ExternalInput").ap()
    wq = nc.dram_tensor("wq", [N_HEADS, P, KC, P], f16,
                        kind="ExternalInput").ap()
    wo = nc.dram_tensor("wo", [DIM, DIM], f16, kind="ExternalInput").ap()
    ktd = nc.dram_tensor("kt", [N_KV_HEADS * P, KV_LEN], f16,
                         kind="ExternalInput").ap()
    vd = nc.dram_tensor("v", [KV_LEN, N_KV_HEADS * P], f16,
                        kind="ExternalInput").ap()
    cqd = nc.dram_tensor("cq", [N_PASS, P, QP], f32, kind="ExternalInput").ap()
    sqd = nc.dram_tensor("sq", [N_PASS, P, QP], f32, kind="ExternalInput").ap()
    ckd = nc.dram_tensor("ck", [P, KV_LEN], f32, kind="ExternalInput").ap()
    skd = nc.dram_tensor("sk", [P, KV_LEN], f32, kind="ExternalInput").ap()
    pmd = nc.dram_tensor("pm", [P, P], f16, kind="ExternalInput").ap()
    ond = nc.dram_tensor("on", [P, P], f16, kind="ExternalInput").ap()
    out = nc.dram_tensor("out", [RQ, DIM], f32, kind="ExternalOutput").ap()

    with tile.TileContext(nc) as tc:
        with ExitStack() as ctx:
            singles = ctx.enter_context(tc.tile_pool(name="singles", bufs=1))
            big = ctx.enter_context(tc.tile_pool(name="big", bufs=2))
            wqp = ctx.enter_context(tc.tile_pool(name="wqp", bufs=3))
            wop = ctx.enter_context(tc.tile_pool(name="wop", bufs=4))
            wk = ctx.enter_context(tc.tile_pool(name="wk", bufs=2))
            esp = ctx.enter_context(tc.tile_pool(name="esp", bufs=6))
            psA = ctx.enter_context(tc.tile_pool(name="psA", bufs=4, space="PSUM"))
            psB = ctx.enter_context(tc.tile_pool(name="psB", bufs=2, space="PSUM"))
            psC = ctx.enter_context(tc.tile_pool(name="psC", bufs=2, space="PSUM"))

            # Resident tensors
            ktf = singles.tile([P, N_KV_HEADS, KV_LEN], f16)   # K^T pre-rope
            kt_sb = singles.tile([P, N_KV_HEADS, KV_LEN], f16)  # K^T post-rope
            v_sb = singles.tile([P, NKT, N_KV_HEADS, P], f16)
            pm_sb = singles.tile([P, P], f16)
            on_sb = singles.tile([P, P], f16)
            ck_sb = singles.tile([P, KV_LEN], f32)
            sk_sb = singles.tile([P, KV_LEN], f32)
            nc.sync.dma_start(ktf, ktd.rearrange("(g d) k -> d g k", d=P))
            nc.sync.dma_start(
                v_sb, vd.rearrange("(kc kin) (g hd) -> kin kc g hd", kin=P, hd=P))
            nc.sync.dma_start(pm_sb, pmd)
            nc.sync.dma_start(on_sb, ond)
            nc.sync.dma_start(ck_sb, ckd)
            nc.sync.dma_start(sk_sb, skd)

            # RoPE on K^T: kt_sb(f16) = ktf*C + swap(ktf)*S
            for g in range(N_KV_HEADS):
                ksw = psB.tile([P, KV_LEN], f32, tag="st")
                nc.tensor.matmul(ksw, pm_sb, ktf[:, g], start=True, stop=True)
                kt1 = wk.tile([P, KV_LEN], f32, tag="vtmp")
                nc.vector.tensor_mul(kt1, ksw, sk_sb)
                kt2 = wk.tile([P, KV_LEN], f32, tag="vtmp2")
                nc.vector.tensor_mul(kt2, ktf[:, g], ck_sb)
                nc.vector.tensor_add(kt_sb[:, g], kt2, kt1)

            for ps in range(N_PASS):
                xt_sb = big.tile([P, KC, QP], f16, tag="xt")
                for kg in range(4):
                    nc.sync.dma_start(
                        xt_sb[:, kg * 8:(kg + 1) * 8, :],
                        xt[kg * 8 * P:(kg + 1) * 8 * P,
                           ps * QP:(ps + 1) * QP].rearrange(
                            "(kc kin) q -> kin kc q", kin=P))
                ot_all = big.tile([P, N_HEADS, QP], f16, tag="ot")
                cq_sb = big.tile([P, QP], f32, tag="cq")
                sq_sb = big.tile([P, QP], f32, tag="sq")
                nc.sync.dma_start(cq_sb, cqd[ps])
                nc.sync.dma_start(sq_sb, sqd[ps])

                # Per-head state carried across the 1-deep software pipeline
                state = {}

                def emit_qt(h):
                    qtp = psA.tile([P, QP], f32, tag="mmout",
                                   name=f"qtp_{ps}_{h}")
                    for kg in range(4):
                        wq_t = wqp.tile([P, KC // 4, P], f16, tag="wq")
                        nc.sync.dma_start(wq_t, wq[h, :, kg * 8:(kg + 1) * 8, :])
                        for k2 in range(8):
                            k = kg * 8 + k2
                            nc.tensor.matmul(qtp, wq_t[:, k2], xt_sb[:, k],
                                             start=(k == 0), stop=(k == KC - 1))
                    qt_raw = wk.tile([P, QP], f16, tag="qt_raw",
                                     name=f"qt_raw_{ps}_{h}")
                    nc.scalar.activation(qt_raw, qtp, Copy)
                    state[h] = (qtp, qt_raw)

                def emit_rope(h):
                    qtp, qt_raw = state[h]
                    qsw = psB.tile([P, QP], f32, tag="st", name=f"qsw_{ps}_{h}")
                    nc.tensor.matmul(qsw, pm_sb, qt_raw, start=True, stop=True)
                    t1 = wk.tile([P, QP], f32, tag="vtmp", name=f"t1_{ps}_{h}")
                    nc.vector.tensor_mul(t1, qsw, sq_sb)
                    qt_rope = wk.tile([P, QP], f32, tag="qt_ropef",
                                      name=f"qt_ropef_{ps}_{h}")
                    nc.vector.tensor_mul(qt_rope, qt_raw, cq_sb)
                    qt_r16 = wk.tile([P, QP], f16, tag="qt_rope",
                                     name=f"qt_rope_{ps}_{h}")
                    nc.vector.tensor_add(qt_r16, qt_rope, t1)
                    state[h] = qt_r16

                def emit_attn(h):
                    qt_r16 = state.pop(h)
                    g = h // REPEATS
                    dnp = psC.tile([1, QP], f32, tag="aux", name=f"dnp_{ps}_{h}")
                    otp = psA.tile([P, QP], f32, tag="mmout",
                                   name=f"otp_{ps}_{h}")
                    ess = []
                    for kt_i in range(NKT):
                        stp = psB.tile([P, QP], f32, tag="st",
                                       name=f"stp_{ps}_{h}_{kt_i}")
                        nc.tensor.matmul(
                            stp, kt_sb[:, g, kt_i * P:(kt_i + 1) * P],
                            qt_r16, start=True, stop=True)
                        es = esp.tile([P, QP], f16, tag="es",
                                      name=f"es_{ps}_{h}_{kt_i}")
                        nc.scalar.activation(es, stp, Exp, scale=SCALE)
                        ess.append(es)
                    for kt_i, es in enumerate(ess):
                        nc.tensor.matmul(dnp, on_sb[:, 0:1], es,
                                         start=(kt_i == 0),
                                         stop=(kt_i == NKT - 1))
                        nc.tensor.matmul(otp, v_sb[:, kt_i, g], es,
                                         start=(kt_i == 0),
                                         stop=(kt_i == NKT - 1))
                    rc = wk.tile([1, QP], f16, tag="rc", name=f"rc_{ps}_{h}")
                    with nc.allow_low_precision(reason="softmax reciprocal"):
                        nc.vector.reciprocal(rc, dnp)
                    rbp = psC.tile([P, QP], f32, tag="aux", name=f"rbp_{ps}_{h}")
                    nc.tensor.matmul(rbp, on_sb[0:1, :], rc,
                                     start=True, stop=True)
                    rb = wk.tile([P, QP], f32, tag="vtmp", name=f"rb_{ps}_{h}")
                    nc.scalar.activation(rb, rbp, Copy)
                    nc.vector.tensor_mul(ot_all[:, h], otp, rb)

                # 1-deep software pipeline over heads
                for h in range(N_HEADS):
                    emit_qt(h)
                    if h > 0:
                        emit_attn(h - 1)
                    emit_rope(h)
                emit_attn(N_HEADS - 1)

                # ---- Phase 3: Y = O @ Wo for this q-pass ----
                for n in range(DIM // 512):
                    yps = [psA.tile([P, 512], f32, tag="mmout",
                                    name=f"y_{ps}_{n}_{m}")
                           for m in range(QP // P)]
                    for h in range(N_HEADS):
                        wo_t = wop.tile([P, 512], f16, tag="wo")
                        nc.sync.dma_start(
                            wo_t, wo[h * P:(h + 1) * P, n * 512:(n + 1) * 512])
                        for m in range(QP // P):
                            nc.tensor.matmul(yps[m],
                                             ot_all[:, h, m * P:(m + 1) * P],
                                             wo_t,
                                             start=(h == 0),
                                             stop=(h == N_HEADS - 1))
                    for m in range(QP // P):
                        ysb = wk.tile([P, 512], f32, tag="y")
                        nc.vector.tensor_copy(ysb, yps[m])
                        r0 = ps * QP + m * P
                        nc.sync.dma_start(
                            out[r0:r0 + P, n * 512:(n + 1) * 512], ysb)

    nc.compile()
    _BUILT = nc
    return nc


def _host_prep(x, xk, xv, Wq, Wo):
    """Build the per-core input maps (shard + layout + dtype cast only)."""
    x = np.asarray(x, dtype=np.float32)
    xk = np.asarray(xk, dtype=np.float32)
    xv = np.asarray(xv, dtype=np.float32)
    Wq = np.asarray(Wq, dtype=np.float32)
    Wo = np.asarray(Wo, dtype=np.float32)
    fp16 = np.float16

    # Shared (same on all cores)
    wq_blk = np.ascontiguousarray(
        Wq.reshape(KC, P, N_HEADS, P).transpose(2, 1, 0, 3)).astype(fp16)
    wo_16 = Wo.astype(fp16)
    pm = np.zeros((P, P), fp16)
    idx = np.arange(0, P, 2)
    pm[idx + 1, idx] = 1.0
    pm[idx, idx + 1] = 1.0
    ones = np.ones((P, P), fp16)

    inv = ROPE_THETA ** (-np.arange(0, HEAD_DIM, 2, dtype=np.float32) / HEAD_DIM)

    def tables(pos):
        ang = pos[None, :].astype(np.float32) * inv[:, None]  # [64, T]
        C = np.repeat(np.cos(ang), 2, axis=0)
        S = np.repeat(np.sin(ang), 2, axis=0)
        S[0::2] *= -1.0
        return np.ascontiguousarray(C), np.ascontiguousarray(S)

    ck, sk = tables(np.arange(KV_LEN))

    in_maps = []
    for c in range(N_CORES):
        r0 = c * RQ
        b = r0 // Q_LEN
        qoff = r0 % Q_LEN
        xt_c = np.ascontiguousarray(x[r0:r0 + RQ].T).astype(fp16)
        kt_c = np.ascontiguousarray(xk[b * KV_LEN:(b + 1) * KV_LEN].T).astype(fp16)
        v_c = xv[b * KV_LEN:(b + 1) * KV_LEN].astype(fp16)
        cq = np.empty((N_PASS, P, QP), np.float32)
        sq = np.empty((N_PASS, P, QP), np.float32)
        for p_i in range(N_PASS):
            Cq, Sq = tables(qoff + p_i * QP + np.arange(QP))
            cq[p_i] = Cq
            sq[p_i] = Sq
        in_maps.append({
            "xt": xt_c, "wq": wq_blk, "wo": wo_16, "kt": kt_c, "v": v_c,
            "cq": cq, "sq": sq, "ck": ck, "sk": sk, "pm": pm, "on": ones,
        })
    return in_maps


def run_sharded(inputs, trace=False, trace_kwargs=None):
    """Build/compile (cached), run on cores 0-7, return (full_out, results)."""
    nc = _build()
    in_maps = _host_prep(inputs["x"], inputs["xk"], inputs["xv"],
                         inputs["Wq"], inputs["Wo"])
    kw = {}
    if trace:
        kw["trace"] = True
        if trace_kwargs:
            kw["trace_kwargs"] = trace_kwargs
    res = run_bass_kernel_spmd(nc, in_maps, core_ids=list(range(N_CORES)), **kw)
    full = np.concatenate([res.results[c]["out"] for c in range(N_CORES)],
                          axis=0)
    return full, res


def kernel(**inputs):
    ns = inputs.get("num_seqs", NUM_SEQS)
    assert int(ns) == NUM_SEQS, f"kernel hardcoded for num_seqs={NUM_SEQS}"
    full, _ = run_sharded(inputs, trace=False)
    return full


# revision 11
# speedup vs baseline: 1.5202x; 1.2254x over previous
"""Trainium2 Bass kernel for varlen GQA cross-attention (4 seqs x 2048 q, 512 kv).

Strategy: data-parallel over query rows. Each of the 8 cores owns 1024 query
rows (half of one sequence) and the full 512-slot KV of that sequence.
No collectives needed.

Per-core dataflow (layouts chosen so no on-device transposes are needed):
  xT [4096,1024] (host pre-transposed)  -> Q^T = Wq.T-chunks x xT   [hd, q]
  RoPE on Q^T / K^T via pair-swap permutation matmul + cos/sin tables
  S^T[k,q] = (K^T chunk).T @ Q^T        (contraction over head_dim)
  expS = exp(S^T * scale)  (ScalarE, PSUM->SBUF)
  denom[1,q] = ones.T @ expS            (partition-dim reduction by matmul)
  O^T[hd,q] = V-chunk.T @ expS          (PSUM accum over k chunks)
  O^T *= broadcast(1/denom)             (broadcast via K=1 ones matmul)
  Y[q,n] = O^T-chunks.T @ Wo-chunks     (accumulate over all 32 heads)

All matmul operands are fp16 (1 cycle/row on the PE, fast weight load that
overlaps with matmuls); every accumulation is fp32 in PSUM, and softmax
intermediates stay fp32 on the vector/scalar engines.

The per-head attention chain (PE -> ACT -> PE -> DVE -> PE) is software
pipelined one head deep so the PE always has the next head's 32 independent
Q-projection matmuls to execute while a head's cross-engine chain resolves.
"""

import sys

if "/opt/trn_rl_repo" not in sys.path:
    sys.path.insert(0, "/opt/trn_rl_repo")

import numpy as np
import ml_dtypes
from contextlib import ExitStack

import concourse.bass as bass
import concourse.tile as tile
import concourse.mybir as mybir
from concourse import bacc
from concourse.bass_utils import run_bass_kernel_spmd

# Problem constants (hardcoded per harness contract)
DIM = 4096
N_HEADS = 32
HEAD_DIM = 128
N_KV_HEADS = 8
REPEATS = N_HEADS // N_KV_HEADS
SCALE = HEAD_DIM ** -0.5
ROPE_THETA = 10000.0
NUM_SEQS = 4
Q_LEN = 2048
KV_LEN = 512
N_CORES = 8
RQ = (NUM_SEQS * Q_LEN) // N_CORES   # 1024 query rows per core
QP = 512                              # q rows per pass
N_PASS = RQ // QP                     # 2
P = 128
KC = DIM // P                         # 32 contraction chunks
NKT = KV_LEN // P                     # 4 kv chunks

f32 = mybir.dt.float32
f16 = mybir.dt.float16
Copy = mybir.ActivationFunctionType.Copy
Exp = mybir.ActivationFunctionType.Exp

_BUILT = None


def _build():
    """Build + compile the per-core Bass program (same NEFF on all 8 cores)."""
    global _BUILT
    if _BUILT is not None:
        return _BUILT

    nc = bacc.Bacc("TRN2", target_bir_lowering=False, debug=False,
                   num_devices=N_CORES)
    xt = nc.dram_tensor("xt", [DIM, RQ], f16, kind="ExternalInput").ap()
    wq = nc.dram_tensor("wq", [N_HEADS, P, KC, P], f16,
                        kind="ExternalInput").ap()
    wo = nc.dram_tensor("wo", [N_HEADS, DIM // 512, P, 512], f16,
                    kind="ExternalInput").ap()
    ktd = nc.dram_tensor("kt", [N_KV_HEADS * P, KV_LEN], f16,
                         kind="ExternalInput").ap()
    vd = nc.dram_tensor("v", [KV_LEN, N_KV_HEADS * P], f16,
                        kind="ExternalInput").ap()
    cqd = nc.dram_tensor("cq", [N_PASS, P, QP], f32, kind="ExternalInput").ap()
    sqd = nc.dram_tensor("sq", [N_PASS, P, QP], f32, kind="ExternalInput").ap()
    ckd = nc.dram_tensor("ck", [P, KV_LEN], f32, kind="ExternalInput").ap()
    skd = nc.dram_tensor("sk", [P, KV_LEN], f32, kind="ExternalInput").ap()
    pmd = nc.dram_tensor("pm", [P, P], f16, kind="ExternalInput").ap()
    ond = nc.dram_tensor("on", [P, P], f16, kind="ExternalInput").ap()
    out = nc.dram_tensor("out", [RQ, DIM], f32, kind="ExternalOutput").ap()

    with tile.TileContext(nc) as tc:
        with ExitStack() as ctx:
            singles = ctx.enter_context(tc.tile_pool(name="singles", bufs=1))
            big = ctx.enter_context(tc.tile_pool(name="big", bufs=2))
            wqp = ctx.enter_context(tc.tile_pool(name="wqp", bufs=2))
            wop = ctx.enter_context(tc.tile_pool(name="wop", bufs=4))
            wk = ctx.enter_context(tc.tile_pool(name="wk", bufs=2))
            esp = ctx.enter_context(tc.tile_pool(name="esp", bufs=5))
            psA = ctx.enter_context(tc.tile_pool(name="psA", bufs=4, space="PSUM"))
            psB = ctx.enter_context(tc.tile_pool(name="psB", bufs=2, space="PSUM"))
            psC = ctx.enter_context(tc.tile_pool(name="psC", bufs=2, space="PSUM"))

            # Resident tensors
            ktf = singles.tile([P, N_KV_HEADS, KV_LEN], f16)   # K^T pre-rope
            kt_sb = singles.tile([P, N_KV_HEADS, KV_LEN], f16)  # K^T post-rope
            v_sb = singles.tile([P, NKT, N_KV_HEADS, P], f16)
            pm_sb = singles.tile([P, P], f16)
            on_sb = singles.tile([P, P], f16)
            ck_sb = singles.tile([P, KV_LEN], f32)
            sk_sb = singles.tile([P, KV_LEN], f32)
            nc.sync.dma_start(ktf, ktd.rearrange("(g d) k -> d g k", d=P))
            nc.sync.dma_start(
                v_sb, vd.rearrange("(kc kin) (g hd) -> kin kc g hd", kin=P, hd=P))
            nc.sync.dma_start(pm_sb, pmd)
            nc.sync.dma_start(on_sb, ond)
            nc.sync.dma_start(ck_sb, ckd)
            nc.sync.dma_start(sk_sb, skd)

            # RoPE on K^T: kt_sb(f16) = ktf*C + swap(ktf)*S
            for g in range(N_KV_HEADS):
                ksw = psB.tile([P, KV_LEN], f32, tag="st")
                nc.tensor.matmul(ksw, pm_sb, ktf[:, g], start=True, stop=True)
                kt1 = wk.tile([P, KV_LEN], f32, tag="vtmp")
                nc.vector.tensor_mul(kt1, ksw, sk_sb)
                kt2 = wk.tile([P, KV_LEN], f32, tag="y")
                nc.vector.tensor_mul(kt2, ktf[:, g], ck_sb)
                nc.vector.tensor_add(kt_sb[:, g], kt2, kt1)

            for ps in range(N_PASS):
                xt_sb = big.tile([P, KC, QP], f16, tag="xt")
                for kg in range(8):
                    nc.sync.dma_start(
                        xt_sb[:, kg * 4:(kg + 1) * 4, :],
                        xt[kg * 4 * P:(kg + 1) * 4 * P,
                           ps * QP:(ps + 1) * QP].rearrange(
                            "(kc kin) q -> kin kc q", kin=P))
                ot_all = big.tile([P, N_HEADS, QP], f16, tag="ot")
                cq_sb = big.tile([P, QP], f32, tag="cq", bufs=1)
                sq_sb = big.tile([P, QP], f32, tag="sq", bufs=1)
                nc.sync.dma_start(cq_sb, cqd[ps])
                nc.sync.dma_start(sq_sb, sqd[ps])

                # Per-head state carried across the software pipeline:
                # stage a (distance 1): scores + exp + denominator + A@V
                # stage b (distance 2): reciprocal + broadcast + normalize
                st_rope = {}
                st_attn = {}

                def emit_qt(h):
                    qtp = psA.tile([P, QP], f32, tag="mmout",
                                   name=f"qtp_{ps}_{h}")
                    wq_t = wqp.tile([P, KC, P], f16, tag="wq")
                    nc.sync.dma_start(wq_t, wq[h])
                    for k in range(KC):
                        nc.tensor.matmul(qtp, wq_t[:, k], xt_sb[:, k],
                                         start=(k == 0), stop=(k == KC - 1))
                    qt_raw = wk.tile([P, QP], f16, tag="qt_raw",
                                     name=f"qt_raw_{ps}_{h}")
                    nc.scalar.activation(qt_raw, qtp, Copy)
                    st_rope[h] = qt_raw

                def emit_rope(h):
                    qt_raw = st_rope.pop(h)
                    qsw = psA.tile([P, QP], f32, tag="mmout",
                                   name=f"qsw_{ps}_{h}")
                    nc.tensor.matmul(qsw, pm_sb, qt_raw, start=True, stop=True)
                    t1 = wk.tile([P, QP], f32, tag="vtmp", name=f"t1_{ps}_{h}")
                    nc.vector.tensor_mul(t1, qsw, sq_sb)
                    qt_rope = wk.tile([P, QP], f32, tag="qt_ropef",
                                      name=f"qt_ropef_{ps}_{h}")
                    nc.vector.tensor_mul(qt_rope, qt_raw, cq_sb)
                    qt_r16 = wk.tile([P, QP], f16, tag="qt_rope",
                                     name=f"qt_rope_{ps}_{h}")
                    nc.vector.tensor_add(qt_r16, qt_rope, t1)
                    st_rope[h] = qt_r16

                def emit_scores(h):
                    qt_r16 = st_rope.pop(h)
                    g = h // REPEATS
                    ess = []
                    for kt_i in range(NKT):
                        stp = psB.tile([P, QP], f32, tag="st",
                                       name=f"stp_{ps}_{h}_{kt_i}")
                        nc.tensor.matmul(
                            stp, kt_sb[:, g, kt_i * P:(kt_i + 1) * P],
                            qt_r16, start=True, stop=True)
                        es = esp.tile([P, QP], f16, tag="es",
                                      name=f"es_{ps}_{h}_{kt_i}")
                        nc.scalar.activation(es, stp, Exp, scale=SCALE)
                        ess.append(es)
                    st_attn[h] = ess

                def emit_dnav(h):
                    ess = st_attn.pop(h)
                    g = h // REPEATS
                    dnp = psC.tile([1, QP], f32, tag="aux", name=f"dnp_{ps}_{h}")
                    otp = psA.tile([P, QP], f32, tag="mmout",
                                   name=f"otp_{ps}_{h}")
                    for kt_i, es in enumerate(ess):
                        nc.tensor.matmul(dnp, on_sb[:, 0:1], es,
                                         start=(kt_i == 0),
                                         stop=(kt_i == NKT - 1))
                        nc.tensor.matmul(otp, v_sb[:, kt_i, g], es,
                                         start=(kt_i == 0),
                                         stop=(kt_i == NKT - 1))
                    st_attn[h] = (dnp, otp)

                def emit_norm(h):
                    dnp, otp = st_attn.pop(h)
                    rc = wk.tile([1, QP], f16, tag="rc", name=f"rc_{ps}_{h}")
                    with nc.allow_low_precision(reason="softmax reciprocal"):
                        nc.vector.reciprocal(rc, dnp)
                    rbp = psC.tile([P, QP], f32, tag="aux", name=f"rbp_{ps}_{h}")
                    nc.tensor.matmul(rbp, on_sb[0:1, :], rc,
                                     start=True, stop=True)
                    rb = wk.tile([P, QP], f32, tag="vtmp", name=f"rb_{ps}_{h}")
                    nc.vector.tensor_copy(rb, rbp)
                    nc.vector.tensor_mul(ot_all[:, h], otp, rb)

                # 2-deep software pipeline over heads
                for h in range(N_HEADS):
                    emit_qt(h)
                    if h > 0:
                        emit_scores(h - 1)
                    if h > 1:
                        emit_norm(h - 2)
                    emit_rope(h)
                    if h > 0:
                        emit_dnav(h - 1)
                emit_scores(N_HEADS - 1)
                emit_norm(N_HEADS - 2)
                emit_dnav(N_HEADS - 1)
                emit_norm(N_HEADS - 1)

                # ---- Phase 3: Y = O @ Wo for this q-pass ----
                for n in range(DIM // 512):
                    yps = [psA.tile([P, 512], f32, tag="mmout",
                                    name=f"y_{ps}_{n}_{m}")
                           for m in range(QP // P)]
                    for h in range(N_HEADS):
                        wo_t = wop.tile([P, 512], f16, tag="wo")
                        nc.sync.dma_start(wo_t, wo[h, n])
                        for m in range(QP // P):
                            nc.tensor.matmul(yps[m],
                                             ot_all[:, h, m * P:(m + 1) * P],
                                             wo_t,
                                             start=(h == 0),
                                             stop=(h == N_HEADS - 1))
                    for m in range(QP // P):
                        ysb = wk.tile([P, 512], f32, tag="y")
                        nc.vector.tensor_copy(ysb, yps[m])
                        r0 = ps * QP + m * P
                        nc.sync.dma_start(
                            out[r0:r0 + P, n * 512:(n + 1) * 512], ysb)

    nc.compile()
    _BUILT = nc
    return nc


def _host_prep(x, xk, xv, Wq, Wo):
    """Build the per-core input maps (shard + layout + dtype cast only)."""
    x = np.asarray(x, dtype=np.float32)
    xk = np.asarray(xk, dtype=np.float32)
    xv = np.asarray(xv, dtype=np.float32)
    Wq = np.asarray(Wq, dtype=np.float32)
    Wo = np.asarray(Wo, dtype=np.float32)
    fp16 = np.float16

    # Shared (same on all cores)
    wq_blk = np.ascontiguousarray(
        Wq.reshape(KC, P, N_HEADS, P).transpose(2, 1, 0, 3)).astype(fp16)
    wo_16 = np.ascontiguousarray(
        Wo.reshape(N_HEADS, P, DIM // 512, 512).transpose(0, 2, 1, 3)
    ).astype(fp16)
    pm = np.zeros((P, P), fp16)
    idx = np.arange(0, P, 2)
    pm[idx + 1, idx] = 1.0
    pm[idx, idx + 1] = 1.0
    ones = np.ones((P, P), fp16)

    inv = ROPE_THETA ** (-np.arange(0, HEAD_DIM, 2, dtype=np.float32) / HEAD_DIM)

    def tables(pos):
        ang = pos[None, :].astype(np.float32) * inv[:, None]  # [64, T]
        C = np.repeat(np.cos(ang), 2, axis=0)
        S = np.repeat(np.sin(ang), 2, axis=0)
        S[0::2] *= -1.0
        return np.ascontiguousarray(C), np.ascontiguousarray(S)

    ck, sk = tables(np.arange(KV_LEN))

    in_maps = []
    for c in range(N_CORES):
        r0 = c * RQ
        b = r0 // Q_LEN
        qoff = r0 % Q_LEN
        xt_c = np.ascontiguousarray(x[r0:r0 + RQ].T).astype(fp16)
        kt_c = np.ascontiguousarray(xk[b * KV_LEN:(b + 1) * KV_LEN].T).astype(fp16)
        v_c = xv[b * KV_LEN:(b + 1) * KV_LEN].astype(fp16)
        cq = np.empty((N_PASS, P, QP), np.float32)
        sq = np.empty((N_PASS, P, QP), np.float32)
        for p_i in range(N_PASS):
            Cq, Sq = tables(qoff + p_i * QP + np.arange(QP))
            cq[p_i] = Cq
            sq[p_i] = Sq
        in_maps.append({
            "xt": xt_c, "wq": wq_blk, "wo": wo_16, "kt": kt_c, "v": v_c,
            "cq": cq, "sq": sq, "ck": ck, "sk": sk, "pm": pm, "on": ones,
        })
    return in_maps


def run_sharded(inputs, trace=False, trace_kwargs=None):
    """Build/compile (cached), run on cores 0-7, return (full_out, results)."""
    nc = _build()
    in_maps = _host_prep(inputs["x"], inputs["xk"], inputs["xv"],
                         inputs["Wq"], inputs["Wo"])
    kw = {}
    if trace:
        kw["trace"] = True
        if trace_kwargs:
            kw["trace_kwargs"] = trace_kwargs
    res = run_bass_kernel_spmd(nc, in_maps, core_ids=list(range(N_CORES)), **kw)
    full = np.concatenate([res.results[c]["out"] for c in range(N_CORES)],
                          axis=0)
    return full, res


def kernel(**inputs):
    ns = inputs.get("num_seqs", NUM_SEQS)
    assert int(ns) == NUM_SEQS, f"kernel hardcoded for num_seqs={NUM_SEQS}"
    full, _ = run_sharded(inputs, trace=False)
    return full


# revision 12
# speedup vs baseline: 1.6582x; 1.0908x over previous
"""Trainium2 Bass kernel for varlen GQA cross-attention (4 seqs x 2048 q, 512 kv).

Strategy: data-parallel over query rows. Each of the 8 cores owns 1024 query
rows (half of one sequence) and the full 512-slot KV of that sequence.
No collectives needed.

Per-core dataflow (layouts chosen so no on-device transposes are needed):
  xT [4096,1024] (host pre-transposed)  -> Q^T = Wq.T-chunks x xT   [hd, q]
  RoPE on Q^T / K^T via pair-swap permutation matmul + cos/sin tables
  S^T[k,q] = (K^T chunk).T @ Q^T        (contraction over head_dim)
  expS = exp(S^T * scale)  (ScalarE, PSUM->SBUF)
  denom[1,q] = ones.T @ expS            (partition-dim reduction by matmul)
  O^T[hd,q] = V-chunk.T @ expS          (PSUM accum over k chunks)
  O^T *= broadcast(1/denom)             (broadcast via K=1 ones matmul)
  Y[q,n] = O^T-chunks.T @ Wo-chunks     (accumulate over all 32 heads)

All matmul operands are fp16 (1 cycle/row on the PE, fast weight load that
overlaps with matmuls); every accumulation is fp32 in PSUM, and softmax
intermediates stay fp32 on the vector/scalar engines.

The per-head attention chain (PE -> ACT -> PE -> DVE -> PE) is software
pipelined one head deep so the PE always has the next head's 32 independent
Q-projection matmuls to execute while a head's cross-engine chain resolves.
"""

import sys

if "/opt/trn_rl_repo" not in sys.path:
    sys.path.insert(0, "/opt/trn_rl_repo")

import numpy as np
import ml_dtypes
from contextlib import ExitStack

import concourse.bass as bass
import concourse.tile as tile
import concourse.mybir as mybir
from concourse import bacc
from concourse.bass_utils import run_bass_kernel_spmd

# Problem constants (hardcoded per harness contract)
DIM = 4096
N_HEADS = 32
HEAD_DIM = 128
N_KV_HEADS = 8
REPEATS = N_HEADS // N_KV_HEADS
SCALE = HEAD_DIM ** -0.5
ROPE_THETA = 10000.0
NUM_SEQS = 4
Q_LEN = 2048
KV_LEN = 512
N_CORES = 8
RQ = (NUM_SEQS * Q_LEN) // N_CORES   # 1024 query rows per core
QP = 512                              # q rows per pass
N_PASS = RQ // QP                     # 2
P = 128
KC = DIM // P                         # 32 contraction chunks
NKT = KV_LEN // P                     # 4 kv chunks

f32 = mybir.dt.float32
f16 = mybir.dt.float16
Copy = mybir.ActivationFunctionType.Copy
Exp = mybir.ActivationFunctionType.Exp

_BUILT = None


def _build():
    """Build + compile the per-core Bass program (same NEFF on all 8 cores)."""
    global _BUILT
    if _BUILT is not None:
        return _BUILT

    nc = bacc.Bacc("TRN2", target_bir_lowering=False, debug=False,
                   num_devices=N_CORES)
    xt = nc.dram_tensor("xt", [DIM, RQ], f16, kind="ExternalInput").ap()
    wq = nc.dram_tensor("wq", [N_HEADS, P, KC, P], f16,
                        kind="ExternalInput").ap()
    wo = nc.dram_tensor("wo", [N_HEADS, DIM // 512, P, 512], f16,
                    kind="ExternalInput").ap()
    ktd = nc.dram_tensor("kt", [N_KV_HEADS * P, KV_LEN], f16,
                         kind="ExternalInput").ap()
    vd = nc.dram_tensor("v", [KV_LEN, N_KV_HEADS * P], f16,
                        kind="ExternalInput").ap()
    cqd = nc.dram_tensor("cq", [N_PASS, P, QP], f32, kind="ExternalInput").ap()
    sqd = nc.dram_tensor("sq", [N_PASS, P, QP], f32, kind="ExternalInput").ap()
    ckd = nc.dram_tensor("ck", [P, KV_LEN], f32, kind="ExternalInput").ap()
    skd = nc.dram_tensor("sk", [P, KV_LEN], f32, kind="ExternalInput").ap()
    pmd = nc.dram_tensor("pm", [P, P], f16, kind="ExternalInput").ap()
    ond = nc.dram_tensor("on", [P, P], f16, kind="ExternalInput").ap()
    out = nc.dram_tensor("out", [RQ, DIM], f32, kind="ExternalOutput").ap()

    with tile.TileContext(nc) as tc:
        with ExitStack() as ctx:
            singles = ctx.enter_context(tc.tile_pool(name="singles", bufs=1))
            big = ctx.enter_context(tc.tile_pool(name="big", bufs=2))
            wqp = ctx.enter_context(tc.tile_pool(name="wqp", bufs=2))
            wop = ctx.enter_context(tc.tile_pool(name="wop", bufs=6))
            wk = ctx.enter_context(tc.tile_pool(name="wk", bufs=2))
            esp = ctx.enter_context(tc.tile_pool(name="esp", bufs=9))
            psA = ctx.enter_context(tc.tile_pool(name="psA", bufs=4, space="PSUM"))
            psB = ctx.enter_context(tc.tile_pool(name="psB", bufs=2, space="PSUM"))
            psC = ctx.enter_context(tc.tile_pool(name="psC", bufs=2, space="PSUM"))

            # Resident tensors
            kt_sb = singles.tile([P, N_KV_HEADS, KV_LEN], f16)  # K^T post-rope
            v_sb = singles.tile([P, NKT, N_KV_HEADS, P], f16)
            pm_sb = singles.tile([P, P], f16)
            on_sb = singles.tile([P, P], f16)
            ck_sb = singles.tile([P, KV_LEN], f32)
            sk_sb = singles.tile([P, KV_LEN], f32)
            nc.sync.dma_start(
                v_sb, vd.rearrange("(kc kin) (g hd) -> kin kc g hd", kin=P, hd=P))
            nc.sync.dma_start(pm_sb, pmd)
            nc.sync.dma_start(on_sb, ond)
            nc.sync.dma_start(ck_sb, ckd)
            nc.sync.dma_start(sk_sb, skd)

            # RoPE on K^T: kt_sb(f16) = ktf*C + swap(ktf)*S
            for g in range(N_KV_HEADS):
                ktf = wk.tile([P, KV_LEN], f16, tag="ktf")
                nc.sync.dma_start(ktf, ktd[g * P:(g + 1) * P, :])
                ksw = psB.tile([P, KV_LEN], f32, tag="st")
                nc.tensor.matmul(ksw, pm_sb, ktf, start=True, stop=True)
                kt1 = wk.tile([P, KV_LEN], f32, tag="vtmp")
                nc.vector.tensor_mul(kt1, ksw, sk_sb)
                kt2 = wk.tile([P, KV_LEN], f32, tag="y")
                nc.vector.tensor_mul(kt2, ktf, ck_sb)
                nc.vector.tensor_add(kt_sb[:, g], kt2, kt1)

            for ps in range(N_PASS):
                xt_sb = big.tile([P, KC, QP], f16, tag="xt")
                for kg in range(8):
                    nc.sync.dma_start(
                        xt_sb[:, kg * 4:(kg + 1) * 4, :],
                        xt[kg * 4 * P:(kg + 1) * 4 * P,
                           ps * QP:(ps + 1) * QP].rearrange(
                            "(kc kin) q -> kin kc q", kin=P))
                ot_all = big.tile([P, N_HEADS, QP], f16, tag="ot")
                cq_sb = big.tile([P, QP], f32, tag="cq", bufs=1)
                sq_sb = big.tile([P, QP], f32, tag="sq", bufs=1)
                nc.sync.dma_start(cq_sb, cqd[ps])
                nc.sync.dma_start(sq_sb, sqd[ps])

                # Per-head state carried across the software pipeline:
                # stage a (distance 1): scores + exp + denominator + A@V
                # stage b (distance 2): reciprocal + broadcast + normalize
                st_rope = {}
                st_attn = {}

                def emit_qt(h):
                    qtp = psA.tile([P, QP], f32, tag="mmout",
                                   name=f"qtp_{ps}_{h}")
                    wq_t = wqp.tile([P, KC, P], f16, tag="wq")
                    nc.sync.dma_start(wq_t, wq[h])
                    for k in range(KC):
                        nc.tensor.matmul(qtp, wq_t[:, k], xt_sb[:, k],
                                         start=(k == 0), stop=(k == KC - 1))
                    qt_raw = wk.tile([P, QP], f16, tag="qt_raw",
                                     name=f"qt_raw_{ps}_{h}")
                    nc.scalar.activation(qt_raw, qtp, Copy)
                    st_rope[h] = qt_raw

                def emit_rope(h):
                    qt_raw = st_rope.pop(h)
                    qsw = psA.tile([P, QP], f32, tag="mmout",
                                   name=f"qsw_{ps}_{h}")
                    nc.tensor.matmul(qsw, pm_sb, qt_raw, start=True, stop=True)
                    t1 = wk.tile([P, QP], f32, tag="vtmp", name=f"t1_{ps}_{h}")
                    nc.vector.tensor_mul(t1, qsw, sq_sb)
                    qt_rope = wk.tile([P, QP], f32, tag="qt_ropef",
                                      name=f"qt_ropef_{ps}_{h}")
                    nc.vector.tensor_mul(qt_rope, qt_raw, cq_sb)
                    qt_r16 = wk.tile([P, QP], f16, tag="qt_rope",
                                     name=f"qt_rope_{ps}_{h}")
                    nc.vector.tensor_add(qt_r16, qt_rope, t1)
                    st_rope[h] = qt_r16

                def emit_scores(h):
                    qt_r16 = st_rope.pop(h)
                    g = h // REPEATS
                    ess = []
                    for kt_i in range(NKT):
                        stp = psB.tile([P, QP], f32, tag="st",
                                       name=f"stp_{ps}_{h}_{kt_i}")
                        nc.tensor.matmul(
                            stp, kt_sb[:, g, kt_i * P:(kt_i + 1) * P],
                            qt_r16, start=True, stop=True)
                        es = esp.tile([P, QP], f16, tag="es",
                                      name=f"es_{ps}_{h}_{kt_i}")
                        nc.scalar.activation(es, stp, Exp, scale=SCALE)
                        ess.append(es)
                    st_attn[h] = ess

                def emit_dnav(h):
                    ess = st_attn.pop(h)
                    g = h // REPEATS
                    dnp = psC.tile([1, QP], f32, tag="aux", name=f"dnp_{ps}_{h}")
                    otp = psA.tile([P, QP], f32, tag="mmout",
                                   name=f"otp_{ps}_{h}")
                    for kt_i, es in enumerate(ess):
                        nc.tensor.matmul(dnp, on_sb[:, 0:1], es,
                                         start=(kt_i == 0),
                                         stop=(kt_i == NKT - 1))
                        nc.tensor.matmul(otp, v_sb[:, kt_i, g], es,
                                         start=(kt_i == 0),
                                         stop=(kt_i == NKT - 1))
                    st_attn[h] = (dnp, otp)

                def emit_norm(h):
                    dnp, otp = st_attn.pop(h)
                    rc = wk.tile([1, QP], f16, tag="rc", name=f"rc_{ps}_{h}")
                    with nc.allow_low_precision(reason="softmax reciprocal"):
                        nc.vector.reciprocal(rc, dnp)
                    rbp = psC.tile([P, QP], f32, tag="aux", name=f"rbp_{ps}_{h}")
                    nc.tensor.matmul(rbp, on_sb[0:1, :], rc,
                                     start=True, stop=True)
                    rb = wk.tile([P, QP], f32, tag="vtmp", name=f"rb_{ps}_{h}")
                    nc.vector.tensor_copy(rb, rbp)
                    nc.vector.tensor_mul(ot_all[:, h], otp, rb)

                # 3-deep software pipeline over heads
                for h in range(N_HEADS):
                    emit_qt(h)
                    if h > 0:
                        emit_scores(h - 1)
                    if h > 1:
                        emit_dnav(h - 2)
                    if h > 2:
                        emit_norm(h - 3)
                    emit_rope(h)
                emit_scores(N_HEADS - 1)
                emit_dnav(N_HEADS - 2)
                emit_norm(N_HEADS - 3)
                emit_dnav(N_HEADS - 1)
                emit_norm(N_HEADS - 2)
                emit_norm(N_HEADS - 1)

                # ---- Phase 3: Y = O @ Wo for this q-pass ----
                for n in range(DIM // 512):
                    yps = [psA.tile([P, 512], f32, tag="mmout",
                                    name=f"y_{ps}_{n}_{m}")
                           for m in range(QP // P)]
                    for h in range(N_HEADS):
                        wo_t = wop.tile([P, 512], f16, tag="wo")
                        nc.sync.dma_start(wo_t, wo[h, n])
                        for m in range(QP // P):
                            nc.tensor.matmul(yps[m],
                                             ot_all[:, h, m * P:(m + 1) * P],
                                             wo_t,
                                             start=(h == 0),
                                             stop=(h == N_HEADS - 1))
                    for m in range(QP // P):
                        ysb = wk.tile([P, 512], f32, tag="y")
                        nc.vector.tensor_copy(ysb, yps[m])
                        r0 = ps * QP + m * P
                        nc.sync.dma_start(
                            out[r0:r0 + P, n * 512:(n + 1) * 512], ysb)

    nc.compile()
    _BUILT = nc
    return nc


def _host_prep(x, xk, xv, Wq, Wo):
    """Build the per-core input maps (shard + layout + dtype cast only)."""
    x = np.asarray(x, dtype=np.float32)
    xk = np.asarray(xk, dtype=np.float32)
    xv = np.asarray(xv, dtype=np.float32)
    Wq = np.asarray(Wq, dtype=np.float32)
    Wo = np.asarray(Wo, dtype=np.float32)
    fp16 = np.float16

    # Shared (same on all cores)
    wq_blk = np.ascontiguousarray(
        Wq.reshape(KC, P, N_HEADS, P).transpose(2, 1, 0, 3)).astype(fp16)
    wo_16 = np.ascontiguousarray(
        Wo.reshape(N_HEADS, P, DIM // 512, 512).transpose(0, 2, 1, 3)
    ).astype(fp16)
    pm = np.zeros((P, P), fp16)
    idx = np.arange(0, P, 2)
    pm[idx + 1, idx] = 1.0
    pm[idx, idx + 1] = 1.0
    ones = np.ones((P, P), fp16)

    inv = ROPE_THETA ** (-np.arange(0, HEAD_DIM, 2, dtype=np.float32) / HEAD_DIM)

    def tables(pos):
        ang = pos[None, :].astype(np.float32) * inv[:, None]  # [64, T]
        C = np.repeat(np.cos(ang), 2, axis=0)
        S = np.repeat(np.sin(ang), 2, axis=0)
        S[0::2] *= -1.0
        return np.ascontiguousarray(C), np.ascontiguousarray(S)

    ck, sk = tables(np.arange(KV_LEN))

    in_maps = []
    for c in range(N_CORES):
        r0 = c * RQ
        b = r0 // Q_LEN
        qoff = r0 % Q_LEN
        xt_c = np.ascontiguousarray(x[r0:r0 + RQ].T).astype(fp16)
        kt_c = np.ascontiguousarray(xk[b * KV_LEN:(b + 1) * KV_LEN].T).astype(fp16)
        v_c = xv[b * KV_LEN:(b + 1) * KV_LEN].astype(fp16)
        cq = np.empty((N_PASS, P, QP), np.float32)
        sq = np.empty((N_PASS, P, QP), np.float32)
        for p_i in range(N_PASS):
            Cq, Sq = tables(qoff + p_i * QP + np.arange(QP))
            cq[p_i] = Cq
            sq[p_i] = Sq
        in_maps.append({
            "xt": xt_c, "wq": wq_blk, "wo": wo_16, "kt": kt_c, "v": v_c,
            "cq": cq, "sq": sq, "ck": ck, "sk": sk, "pm": pm, "on": ones,
        })
    return in_maps


def run_sharded(inputs, trace=False, trace_kwargs=None):
    """Build/compile (cached), run on cores 0-7, return (full_out, results)."""
    nc = _build()
    in_maps = _host_prep(inputs["x"], inputs["xk"], inputs["xv"],
                         inputs["Wq"], inputs["Wo"])
    kw = {}
    if trace:
        kw["trace"] = True
        if trace_kwargs:
            kw["trace_kwargs"] = trace_kwargs
    res = run_bass_kernel_spmd(nc, in_maps, core_ids=list(range(N_CORES)), **kw)
    full = np.concatenate([res.results[c]["out"] for c in range(N_CORES)],
                          axis=0)
    return full, res


def kernel(**inputs):
    ns = inputs.get("num_seqs", NUM_SEQS)
    assert int(ns) == NUM_SEQS, f"kernel hardcoded for num_seqs={NUM_SEQS}"
    full, _ = run_sharded(inputs, trace=False)
    return full


# revision 13
# speedup vs baseline: 1.7563x; 1.0592x over previous
"""Trainium2 Bass kernel for varlen GQA cross-attention (4 seqs x 2048 q, 512 kv).

Strategy: data-parallel over query rows. Each of the 8 cores owns 1024 query
rows (half of one sequence) and the full 512-slot KV of that sequence.
No collectives needed.

Per-core dataflow (layouts chosen so no on-device transposes are needed):
  xT [4096,1024] (host pre-transposed)  -> Q^T = Wq.T-chunks x xT   [hd, q]
  RoPE on Q^T / K^T via pair-swap permutation matmul + cos/sin tables
  S^T[k,q] = (K^T chunk).T @ Q^T        (contraction over head_dim)
  expS = exp(S^T * scale)  (ScalarE, PSUM->SBUF)
  denom[1,q] = ones.T @ expS            (partition-dim reduction by matmul)
  O^T[hd,q] = V-chunk.T @ expS          (PSUM accum over k chunks)
  O^T *= broadcast(1/denom)             (broadcast via K=1 ones matmul)
  Y[q,n] = O^T-chunks.T @ Wo-chunks     (accumulate over all 32 heads)

All matmul operands are fp16 (1 cycle/row on the PE, fast weight load that
overlaps with matmuls); every accumulation is fp32 in PSUM, and softmax
intermediates stay fp32 on the vector/scalar engines.

The per-head attention chain (PE -> ACT -> PE -> DVE -> PE) is software
pipelined one head deep so the PE always has the next head's 32 independent
Q-projection matmuls to execute while a head's cross-engine chain resolves.
"""

import sys

if "/opt/trn_rl_repo" not in sys.path:
    sys.path.insert(0, "/opt/trn_rl_repo")

import numpy as np
import ml_dtypes
from contextlib import ExitStack

import concourse.bass as bass
import concourse.tile as tile
import concourse.mybir as mybir
from concourse import bacc
from concourse.bass_utils import run_bass_kernel_spmd

# Problem constants (hardcoded per harness contract)
DIM = 4096
N_HEADS = 32
HEAD_DIM = 128
N_KV_HEADS = 8
REPEATS = N_HEADS // N_KV_HEADS
SCALE = HEAD_DIM ** -0.5
ROPE_THETA = 10000.0
NUM_SEQS = 4
Q_LEN = 2048
KV_LEN = 512
N_CORES = 8
RQ = (NUM_SEQS * Q_LEN) // N_CORES   # 1024 query rows per core
QP = 512                              # q rows per pass
N_PASS = RQ // QP                     # 2
P = 128
KC = DIM // P                         # 32 contraction chunks
NKT = KV_LEN // P                     # 4 kv chunks

f32 = mybir.dt.float32
f16 = mybir.dt.float16
Copy = mybir.ActivationFunctionType.Copy
Exp = mybir.ActivationFunctionType.Exp

_BUILT = None


def _build():
    """Build + compile the per-core Bass program (same NEFF on all 8 cores)."""
    global _BUILT
    if _BUILT is not None:
        return _BUILT

    nc = bacc.Bacc("TRN2", target_bir_lowering=False, debug=False,
                   num_devices=N_CORES)
    xt = nc.dram_tensor("xt", [DIM, RQ], f16, kind="ExternalInput").ap()
    wq = nc.dram_tensor("wq", [N_HEADS, P, KC, P], f16,
                        kind="ExternalInput").ap()
    wo = nc.dram_tensor("wo", [N_HEADS, DIM // 512, P, 512], f16,
                    kind="ExternalInput").ap()
    ktd = nc.dram_tensor("kt", [N_KV_HEADS * P, KV_LEN], f16,
                         kind="ExternalInput").ap()
    vd = nc.dram_tensor("v", [KV_LEN, N_KV_HEADS * P], f16,
                        kind="ExternalInput").ap()
    cqd = nc.dram_tensor("cq", [N_PASS, P, QP], f16, kind="ExternalInput").ap()
    sqd = nc.dram_tensor("sq", [N_PASS, P, QP], f16, kind="ExternalInput").ap()
    ckd = nc.dram_tensor("ck", [P, KV_LEN], f16, kind="ExternalInput").ap()
    skd = nc.dram_tensor("sk", [P, KV_LEN], f16, kind="ExternalInput").ap()
    pmd = nc.dram_tensor("pm", [P, P], f16, kind="ExternalInput").ap()
    ond = nc.dram_tensor("on", [P, P], f16, kind="ExternalInput").ap()
    out = nc.dram_tensor("out", [RQ, DIM], f32, kind="ExternalOutput").ap()

    with tile.TileContext(nc) as tc:
        with ExitStack() as ctx:
            singles = ctx.enter_context(tc.tile_pool(name="singles", bufs=1))
            big = ctx.enter_context(tc.tile_pool(name="big", bufs=2))
            wqp = ctx.enter_context(tc.tile_pool(name="wqp", bufs=3))
            wop = ctx.enter_context(tc.tile_pool(name="wop", bufs=5))
            wk = ctx.enter_context(tc.tile_pool(name="wk", bufs=2))
            esp = ctx.enter_context(tc.tile_pool(name="esp", bufs=8))
            psA = ctx.enter_context(tc.tile_pool(name="psA", bufs=4, space="PSUM"))
            psB = ctx.enter_context(tc.tile_pool(name="psB", bufs=2, space="PSUM"))
            psC = ctx.enter_context(tc.tile_pool(name="psC", bufs=2, space="PSUM"))

            # Resident tensors
            kt_sb = singles.tile([P, N_KV_HEADS, KV_LEN], f16)  # K^T post-rope
            v_sb = singles.tile([P, NKT, N_KV_HEADS, P], f16)
            pm_sb = singles.tile([P, P], f16)
            on_sb = singles.tile([P, P], f16)
            ck_sb = singles.tile([P, KV_LEN], f16)
            sk_sb = singles.tile([P, KV_LEN], f16)
            nc.sync.dma_start(
                v_sb, vd.rearrange("(kc kin) (g hd) -> kin kc g hd", kin=P, hd=P))
            nc.sync.dma_start(pm_sb, pmd)
            nc.sync.dma_start(on_sb, ond)
            nc.sync.dma_start(ck_sb, ckd)
            nc.sync.dma_start(sk_sb, skd)

            # RoPE on K^T: kt_sb(f16) = ktf*C + swap(ktf)*S
            for g in range(N_KV_HEADS):
                ktf = wk.tile([P, KV_LEN], f16, tag="ktf")
                nc.sync.dma_start(ktf, ktd[g * P:(g + 1) * P, :])
                ksw = psB.tile([P, KV_LEN], f32, tag="st")
                nc.tensor.matmul(ksw, pm_sb, ktf, start=True, stop=True)
                kt1 = wk.tile([P, KV_LEN], f32, tag="vtmp")
                nc.vector.tensor_mul(kt1, ksw, sk_sb)
                kt2 = wk.tile([P, KV_LEN], f32, tag="y")
                nc.vector.tensor_mul(kt2, ktf, ck_sb)
                nc.vector.tensor_add(kt_sb[:, g], kt2, kt1)

            for ps in range(N_PASS):
                xt_sb = big.tile([P, KC, QP], f16, tag="xt")
                for kg in range(8):
                    nc.sync.dma_start(
                        xt_sb[:, kg * 4:(kg + 1) * 4, :],
                        xt[kg * 4 * P:(kg + 1) * 4 * P,
                           ps * QP:(ps + 1) * QP].rearrange(
                            "(kc kin) q -> kin kc q", kin=P))
                ot_all = big.tile([P, N_HEADS, QP], f16, tag="ot")
                cq_sb = big.tile([P, QP], f16, tag="cq", bufs=1)
                sq_sb = big.tile([P, QP], f16, tag="sq", bufs=1)
                nc.sync.dma_start(cq_sb, cqd[ps])
                nc.sync.dma_start(sq_sb, sqd[ps])

                # Per-head state carried across the software pipeline:
                # stage a (distance 1): scores + exp + denominator + A@V
                # stage b (distance 2): reciprocal + broadcast + normalize
                st_rope = {}
                st_attn = {}

                def emit_qt(h):
                    qtp = psA.tile([P, QP], f32, tag="mmout",
                                   name=f"qtp_{ps}_{h}")
                    wq_t = wqp.tile([P, KC, P], f16, tag="wq")
                    nc.sync.dma_start(wq_t, wq[h])
                    for k in range(KC):
                        nc.tensor.matmul(qtp, wq_t[:, k], xt_sb[:, k],
                                         start=(k == 0), stop=(k == KC - 1))
                    qt_raw = wk.tile([P, QP], f16, tag="qt_raw",
                                     name=f"qt_raw_{ps}_{h}")
                    nc.scalar.activation(qt_raw, qtp, Copy)
                    st_rope[h] = qt_raw

                def emit_rope(h):
                    qt_raw = st_rope.pop(h)
                    qsw = psA.tile([P, QP], f32, tag="mmout",
                                   name=f"qsw_{ps}_{h}")
                    nc.tensor.matmul(qsw, pm_sb, qt_raw, start=True, stop=True)
                    t1 = wk.tile([P, QP], f32, tag="vtmp", name=f"t1_{ps}_{h}")
                    nc.vector.tensor_mul(t1, qsw, sq_sb)
                    qt_r16 = wk.tile([P, QP], f16, tag="qt_rope",
                                     name=f"qt_rope_{ps}_{h}")
                    nc.vector.tensor_mul(qt_r16, qt_raw, cq_sb)
                    nc.vector.tensor_add(qt_r16, qt_r16, t1)
                    st_rope[h] = qt_r16

                def emit_scores(h):
                    qt_r16 = st_rope.pop(h)
                    g = h // REPEATS
                    ess = []
                    for kt_i in range(NKT):
                        stp = psB.tile([P, QP], f32, tag="st",
                                       name=f"stp_{ps}_{h}_{kt_i}")
                        nc.tensor.matmul(
                            stp, kt_sb[:, g, kt_i * P:(kt_i + 1) * P],
                            qt_r16, start=True, stop=True)
                        es = esp.tile([P, QP], f16, tag="es",
                                      name=f"es_{ps}_{h}_{kt_i}")
                        nc.scalar.activation(es, stp, Exp, scale=SCALE)
                        ess.append(es)
                    st_attn[h] = ess

                def emit_dnav(h):
                    ess = st_attn.pop(h)
                    g = h // REPEATS
                    # Full ones block: every psum partition gets the denominator
                    # (same PE cost; kills the later broadcast matmul).
                    dnp = psC.tile([P, QP], f32, tag="aux", name=f"dnp_{ps}_{h}")
                    otp = psA.tile([P, QP], f32, tag="mmout",
                                   name=f"otp_{ps}_{h}")
                    for kt_i, es in enumerate(ess):
                        nc.tensor.matmul(dnp, on_sb, es,
                                         start=(kt_i == 0),
                                         stop=(kt_i == NKT - 1))
                        nc.tensor.matmul(otp, v_sb[:, kt_i, g], es,
                                         start=(kt_i == 0),
                                         stop=(kt_i == NKT - 1))
                    st_attn[h] = (dnp, otp)

                def emit_norm(h):
                    dnp, otp = st_attn.pop(h)
                    rc = wk.tile([P, QP], f16, tag="rc", name=f"rc_{ps}_{h}")
                    with nc.allow_low_precision(reason="softmax reciprocal"):
                        nc.vector.reciprocal(rc, dnp)
                    nc.vector.tensor_mul(ot_all[:, h], otp, rc)

                # 3-deep software pipeline over heads
                for h in range(N_HEADS):
                    emit_qt(h)
                    if h > 0:
                        emit_scores(h - 1)
                    if h > 1:
                        emit_dnav(h - 2)
                    if h > 2:
                        emit_norm(h - 3)
                    emit_rope(h)
                emit_scores(N_HEADS - 1)
                emit_dnav(N_HEADS - 2)
                emit_norm(N_HEADS - 3)
                emit_dnav(N_HEADS - 1)
                emit_norm(N_HEADS - 2)
                emit_norm(N_HEADS - 1)

                # ---- Phase 3: Y = O @ Wo for this q-pass ----
                for n in range(DIM // 512):
                    yps = [psA.tile([P, 512], f32, tag="mmout",
                                    name=f"y_{ps}_{n}_{m}")
                           for m in range(QP // P)]
                    for h in range(N_HEADS):
                        wo_t = wop.tile([P, 512], f16, tag="wo")
                        nc.sync.dma_start(wo_t, wo[h, n])
                        for m in range(QP // P):
                            nc.tensor.matmul(yps[m],
                                             ot_all[:, h, m * P:(m + 1) * P],
                                             wo_t,
                                             start=(h == 0),
                                             stop=(h == N_HEADS - 1))
                    for m in range(QP // P):
                        ysb = wk.tile([P, 512], f32, tag="y")
                        if m % 2 == 0:
                            nc.vector.tensor_copy(ysb, yps[m])
                        else:
                            nc.scalar.activation(ysb, yps[m], Copy)
                        r0 = ps * QP + m * P
                        nc.sync.dma_start(
                            out[r0:r0 + P, n * 512:(n + 1) * 512], ysb)

    nc.compile()
    _BUILT = nc
    return nc


def _host_prep(x, xk, xv, Wq, Wo):
    """Build the per-core input maps (shard + layout + dtype cast only)."""
    x = np.asarray(x, dtype=np.float32)
    xk = np.asarray(xk, dtype=np.float32)
    xv = np.asarray(xv, dtype=np.float32)
    Wq = np.asarray(Wq, dtype=np.float32)
    Wo = np.asarray(Wo, dtype=np.float32)
    fp16 = np.float16

    # Shared (same on all cores)
    wq_blk = np.ascontiguousarray(
        Wq.reshape(KC, P, N_HEADS, P).transpose(2, 1, 0, 3)).astype(fp16)
    wo_16 = np.ascontiguousarray(
        Wo.reshape(N_HEADS, P, DIM // 512, 512).transpose(0, 2, 1, 3)
    ).astype(fp16)
    pm = np.zeros((P, P), fp16)
    idx = np.arange(0, P, 2)
    pm[idx + 1, idx] = 1.0
    pm[idx, idx + 1] = 1.0
    ones = np.ones((P, P), fp16)

    inv = ROPE_THETA ** (-np.arange(0, HEAD_DIM, 2, dtype=np.float32) / HEAD_DIM)

    def tables(pos):
        ang = pos[None, :].astype(np.float32) * inv[:, None]  # [64, T]
        C = np.repeat(np.cos(ang), 2, axis=0)
        S = np.repeat(np.sin(ang), 2, axis=0)
        S[0::2] *= -1.0
        return (np.ascontiguousarray(C).astype(np.float16),
                np.ascontiguousarray(S).astype(np.float16))

    ck, sk = tables(np.arange(KV_LEN))

    in_maps = []
    for c in range(N_CORES):
        r0 = c * RQ
        b = r0 // Q_LEN
        qoff = r0 % Q_LEN
        xt_c = np.ascontiguousarray(x[r0:r0 + RQ].T).astype(fp16)
        kt_c = np.ascontiguousarray(xk[b * KV_LEN:(b + 1) * KV_LEN].T).astype(fp16)
        v_c = xv[b * KV_LEN:(b + 1) * KV_LEN].astype(fp16)
        cq = np.empty((N_PASS, P, QP), np.float16)
        sq = np.empty((N_PASS, P, QP), np.float16)
        for p_i in range(N_PASS):
            Cq, Sq = tables(qoff + p_i * QP + np.arange(QP))
            cq[p_i] = Cq
            sq[p_i] = Sq
        in_maps.append({
            "xt": xt_c, "wq": wq_blk, "wo": wo_16, "kt": kt_c, "v": v_c,
            "cq": cq, "sq": sq, "ck": ck, "sk": sk, "pm": pm, "on": ones,
        })
    return in_maps


def run_sharded(inputs, trace=False, trace_kwargs=None):
    """Build/compile (cached), run on cores 0-7, return (full_out, results)."""
    nc = _build()
    in_maps = _host_prep(inputs["x"], inputs["xk"], inputs["xv"],
                         inputs["Wq"], inputs["Wo"])
    kw = {}
    if trace:
        kw["trace"] = True
        if trace_kwargs:
            kw["trace_kwargs"] = trace_kwargs
    res = run_bass_kernel_spmd(nc, in_maps, core_ids=list(range(N_CORES)), **kw)
    full = np.concatenate([res.results[c]["out"] for c in range(N_CORES)],
                          axis=0)
    return full, res


def kernel(**inputs):
    ns = inputs.get("num_seqs", NUM_SEQS)
    assert int(ns) == NUM_SEQS, f"kernel hardcoded for num_seqs={NUM_SEQS}"
    full, _ = run_sharded(inputs, trace=False)
    return full


# revision 15
# speedup vs baseline: 1.7915x; 1.0200x over previous
"""Trainium2 Bass kernel for varlen GQA cross-attention (4 seqs x 2048 q, 512 kv).

Strategy: data-parallel over query rows. Each of the 8 cores owns 1024 query
rows (half of one sequence) and the full 512-slot KV of that sequence.
No collectives needed.

Per-core dataflow (layouts chosen so no on-device transposes are needed):
  xT [4096,1024] (host pre-transposed)  -> Q^T = Wq.T-chunks x xT   [hd, q]
  RoPE on Q^T / K^T via pair-swap permutation matmul + cos/sin tables
  S^T[k,q] = (K^T chunk).T @ Q^T        (contraction over head_dim)
  expS = exp(S^T * scale)  (ScalarE, PSUM->SBUF)
  denom[1,q] = ones.T @ expS            (partition-dim reduction by matmul)
  O^T[hd,q] = V-chunk.T @ expS          (PSUM accum over k chunks)
  O^T *= broadcast(1/denom)             (broadcast via K=1 ones matmul)
  Y[q,n] = O^T-chunks.T @ Wo-chunks     (accumulate over all 32 heads)

All matmul operands are fp16 (1 cycle/row on the PE, fast weight load that
overlaps with matmuls); every accumulation is fp32 in PSUM, and softmax
intermediates stay fp32 on the vector/scalar engines.

The per-head attention chain (PE -> ACT -> PE -> DVE -> PE) is software
pipelined one head deep so the PE always has the next head's 32 independent
Q-projection matmuls to execute while a head's cross-engine chain resolves.
"""

import sys

if "/opt/trn_rl_repo" not in sys.path:
    sys.path.insert(0, "/opt/trn_rl_repo")

import numpy as np
import ml_dtypes
from contextlib import ExitStack

import concourse.bass as bass
import concourse.tile as tile
import concourse.mybir as mybir
from concourse import bacc
from concourse.bass_utils import run_bass_kernel_spmd

# Problem constants (hardcoded per harness contract)
DIM = 4096
N_HEADS = 32
HEAD_DIM = 128
N_KV_HEADS = 8
REPEATS = N_HEADS // N_KV_HEADS
SCALE = HEAD_DIM ** -0.5
ROPE_THETA = 10000.0
NUM_SEQS = 4
Q_LEN = 2048
KV_LEN = 512
N_CORES = 8
RQ = (NUM_SEQS * Q_LEN) // N_CORES   # 1024 query rows per core
QP = 512                              # q rows per pass
N_PASS = RQ // QP                     # 2
P = 128
KC = DIM // P                         # 32 contraction chunks
NKT = KV_LEN // P                     # 4 kv chunks

f32 = mybir.dt.float32
f16 = mybir.dt.float16
Copy = mybir.ActivationFunctionType.Copy
Exp = mybir.ActivationFunctionType.Exp

_BUILT = None


def _build():
    """Build + compile the per-core Bass program (same NEFF on all 8 cores)."""
    global _BUILT
    if _BUILT is not None:
        return _BUILT

    nc = bacc.Bacc("TRN2", target_bir_lowering=False, debug=False,
                   num_devices=N_CORES)
    xt = nc.dram_tensor("xt", [DIM, RQ], f16, kind="ExternalInput").ap()
    wq = nc.dram_tensor("wq", [N_HEADS, P, KC, P], f16,
                        kind="ExternalInput").ap()
    wo = nc.dram_tensor("wo", [N_HEADS, DIM // 512, P, 512], f16,
                    kind="ExternalInput").ap()
    ktd = nc.dram_tensor("kt", [N_KV_HEADS * P, KV_LEN], f16,
                         kind="ExternalInput").ap()
    vd = nc.dram_tensor("v", [KV_LEN, N_KV_HEADS * P], f16,
                        kind="ExternalInput").ap()
    cqd = nc.dram_tensor("cq", [N_PASS, P, QP], f16, kind="ExternalInput").ap()
    sqd = nc.dram_tensor("sq", [N_PASS, P, QP], f16, kind="ExternalInput").ap()
    ckd = nc.dram_tensor("ck", [P, KV_LEN], f16, kind="ExternalInput").ap()
    skd = nc.dram_tensor("sk", [P, KV_LEN], f16, kind="ExternalInput").ap()
    pmd = nc.dram_tensor("pm", [P, P], f16, kind="ExternalInput").ap()
    ond = nc.dram_tensor("on", [P, P], f16, kind="ExternalInput").ap()
    out = nc.dram_tensor("out", [RQ, DIM], f32, kind="ExternalOutput").ap()

    with tile.TileContext(nc) as tc:
        with ExitStack() as ctx:
            singles = ctx.enter_context(tc.tile_pool(name="singles", bufs=1))
            big = ctx.enter_context(tc.tile_pool(name="big", bufs=2))
            wqp = ctx.enter_context(tc.tile_pool(name="wqp", bufs=3))
            wop = ctx.enter_context(tc.tile_pool(name="wop", bufs=5))
            wk = ctx.enter_context(tc.tile_pool(name="wk", bufs=2))
            esp = ctx.enter_context(tc.tile_pool(name="esp", bufs=8))
            psA = ctx.enter_context(tc.tile_pool(name="psA", bufs=4, space="PSUM"))
            ph12 = ExitStack()
            psB = ph12.enter_context(tc.tile_pool(name="psB", bufs=2, space="PSUM"))
            psC = ph12.enter_context(tc.tile_pool(name="psC", bufs=2, space="PSUM"))

            # Resident tensors
            kt_sb = singles.tile([P, N_KV_HEADS, KV_LEN], f16)  # K^T post-rope
            v_sb = singles.tile([P, NKT, N_KV_HEADS, P], f16)
            pm_sb = singles.tile([P, P], f16)
            on_sb = singles.tile([P, P], f16)
            ck_sb = singles.tile([P, KV_LEN], f16)
            sk_sb = singles.tile([P, KV_LEN], f16)
            nc.sync.dma_start(
                v_sb, vd.rearrange("(kc kin) (g hd) -> kin kc g hd", kin=P, hd=P))
            nc.sync.dma_start(pm_sb, pmd)
            nc.sync.dma_start(on_sb, ond)
            nc.sync.dma_start(ck_sb, ckd)
            nc.sync.dma_start(sk_sb, skd)

            # RoPE on K^T: kt_sb(f16) = ktf*C + swap(ktf)*S
            for g in range(N_KV_HEADS):
                ktf = wk.tile([P, KV_LEN], f16, tag="ktf")
                nc.sync.dma_start(ktf, ktd[g * P:(g + 1) * P, :])
                ksw = psB.tile([P, KV_LEN], f32, tag="st")
                nc.tensor.matmul(ksw, pm_sb, ktf, start=True, stop=True)
                kt1 = wk.tile([P, KV_LEN], f32, tag="vtmp")
                nc.vector.tensor_mul(kt1, ksw, sk_sb)
                kt2 = wk.tile([P, KV_LEN], f32, tag="y")
                nc.vector.tensor_mul(kt2, ktf, ck_sb)
                nc.vector.tensor_add(kt_sb[:, g], kt2, kt1)

            # Both q-passes resident; units are (head, pass) so each Wq
            # head block is loaded once and serves both passes.
            xts, ots, cqs, sqs = [], [], [], []
            for ps in range(N_PASS):
                xt_sb = big.tile([P, KC, QP], f16, tag="xt",
                                 name=f"xt_{ps}")
                for kg in range(8):
                    nc.sync.dma_start(
                        xt_sb[:, kg * 4:(kg + 1) * 4, :],
                        xt[kg * 4 * P:(kg + 1) * 4 * P,
                           ps * QP:(ps + 1) * QP].rearrange(
                            "(kc kin) q -> kin kc q", kin=P))
                ot_all = big.tile([P, N_HEADS, QP], f16, tag="ot",
                                  name=f"ot_{ps}")
                cq_sb = big.tile([P, QP], f16, tag="cq", name=f"cq_{ps}")
                sq_sb = big.tile([P, QP], f16, tag="sq", name=f"sq_{ps}")
                nc.sync.dma_start(cq_sb, cqd[ps])
                nc.sync.dma_start(sq_sb, sqd[ps])
                xts.append(xt_sb); ots.append(ot_all)
                cqs.append(cq_sb); sqs.append(sq_sb)

            NU = N_HEADS * N_PASS
            st_rope = {}
            st_attn = {}
            wq_tiles = {}

            def emit_qt(u):
                h, ps = u // N_PASS, u % N_PASS
                qtp = psA.tile([P, QP], f32, tag="mmout", name=f"qtp_{u}")
                if ps == 0:
                    wq_t = wqp.tile([P, KC, P], f16, tag="wq")
                    nc.sync.dma_start(wq_t, wq[h])
                    wq_tiles[h] = wq_t
                else:
                    wq_t = wq_tiles.pop(h)
                for k in range(KC):
                    nc.tensor.matmul(qtp, wq_t[:, k], xts[ps][:, k],
                                     start=(k == 0), stop=(k == KC - 1))
                qt_raw = wk.tile([P, QP], f16, tag="qt_raw",
                                 name=f"qt_raw_{u}")
                nc.scalar.activation(qt_raw, qtp, Copy)
                st_rope[u] = qt_raw

            def emit_rope(u):
                h, ps = u // N_PASS, u % N_PASS
                qt_raw = st_rope.pop(u)
                qsw = psA.tile([P, QP], f32, tag="mmout", name=f"qsw_{u}")
                nc.tensor.matmul(qsw, pm_sb, qt_raw, start=True, stop=True)
                t1 = wk.tile([P, QP], f32, tag="vtmp", name=f"t1_{u}")
                nc.vector.tensor_mul(t1, qsw, sqs[ps])
                qt_r16 = wk.tile([P, QP], f16, tag="qt_rope",
                                 name=f"qt_rope_{u}")
                nc.vector.tensor_mul(qt_r16, qt_raw, cqs[ps])
                nc.vector.tensor_add(qt_r16, qt_r16, t1)
                st_rope[u] = qt_r16

            def emit_scores(u):
                h = u // N_PASS
                qt_r16 = st_rope.pop(u)
                g = h // REPEATS
                ess = []
                for kt_i in range(NKT):
                    stp = psB.tile([P, QP], f32, tag="st",
                                   name=f"stp_{u}_{kt_i}")
                    nc.tensor.matmul(
                        stp, kt_sb[:, g, kt_i * P:(kt_i + 1) * P],
                        qt_r16, start=True, stop=True)
                    es = esp.tile([P, QP], f16, tag="es",
                                  name=f"es_{u}_{kt_i}")
                    nc.scalar.activation(es, stp, Exp, scale=SCALE)
                    ess.append(es)
                st_attn[u] = ess

            def emit_dnav(u):
                h = u // N_PASS
                ess = st_attn.pop(u)
                g = h // REPEATS
                # Full ones block: every psum partition gets the denominator
                # (same PE cost; no broadcast needed afterwards).
                dnp = psC.tile([P, QP], f32, tag="aux", name=f"dnp_{u}")
                otp = psA.tile([P, QP], f32, tag="mmout", name=f"otp_{u}")
                for kt_i, es in enumerate(ess):
                    nc.tensor.matmul(dnp, on_sb, es,
                                     start=(kt_i == 0),
                                     stop=(kt_i == NKT - 1))
                    nc.tensor.matmul(otp, v_sb[:, kt_i, g], es,
                                     start=(kt_i == 0),
                                     stop=(kt_i == NKT - 1))
                st_attn[u] = (dnp, otp)

            def emit_norm(u):
                h, ps = u // N_PASS, u % N_PASS
                dnp, otp = st_attn.pop(u)
                rc = wk.tile([P, QP], f16, tag="rc", name=f"rc_{u}")
                with nc.allow_low_precision(reason="softmax reciprocal"):
                    nc.vector.reciprocal(rc, dnp)
                nc.vector.tensor_mul(ots[ps][:, h], otp, rc)

            # 3-deep software pipeline over (head, pass) units
            for u in range(NU):
                emit_qt(u)
                if u > 0:
                    emit_scores(u - 1)
                if u > 1:
                    emit_dnav(u - 2)
                if u > 2:
                    emit_norm(u - 3)
                emit_rope(u)
            emit_scores(NU - 1)
            emit_dnav(NU - 2)
            emit_norm(NU - 3)
            emit_dnav(NU - 1)
            emit_norm(NU - 2)
            emit_norm(NU - 1)

            # ---- Phase 3: Y = O @ Wo, both passes share each Wo block ----
            ph12.close()
            with tc.tile_pool(name="psY", bufs=4, space="PSUM") as psY:
                for n in range(DIM // 512):
                    yps = [psA.tile([P, 512], f32, tag="mmout",
                                    name=f"yA_{n}_{m}")
                           for m in range(QP // P)]
                    yps += [psY.tile([P, 512], f32, tag="y2",
                                     name=f"yB_{n}_{m}")
                            for m in range(QP // P)]
                    for h in range(N_HEADS):
                        wo_t = wop.tile([P, 512], f16, tag="wo")
                        nc.sync.dma_start(wo_t, wo[h, n])
                        for ps in range(N_PASS):
                            for m in range(QP // P):
                                nc.tensor.matmul(
                                    yps[ps * 4 + m],
                                    ots[ps][:, h, m * P:(m + 1) * P],
                                    wo_t,
                                    start=(h == 0),
                                    stop=(h == N_HEADS - 1))
                    for ps in range(N_PASS):
                        for m in range(QP // P):
                            ysb = wk.tile([P, 512], f32, tag="y")
                            if m % 2 == 0:
                                nc.vector.tensor_copy(ysb, yps[ps * 4 + m])
                            else:
                                nc.scalar.activation(ysb, yps[ps * 4 + m], Copy)
                            r0 = ps * QP + m * P
                            nc.sync.dma_start(
                                out[r0:r0 + P, n * 512:(n + 1) * 512], ysb)

    nc.compile()
    _BUILT = nc
    return nc


def _host_prep(x, xk, xv, Wq, Wo):
    """Build the per-core input maps (shard + layout + dtype cast only)."""
    x = np.asarray(x, dtype=np.float32)
    xk = np.asarray(xk, dtype=np.float32)
    xv = np.asarray(xv, dtype=np.float32)
    Wq = np.asarray(Wq, dtype=np.float32)
    Wo = np.asarray(Wo, dtype=np.float32)
    fp16 = np.float16

    # Shared (same on all cores)
    wq_blk = np.ascontiguousarray(
        Wq.reshape(KC, P, N_HEADS, P).transpose(2, 1, 0, 3)).astype(fp16)
    wo_16 = np.ascontiguousarray(
        Wo.reshape(N_HEADS, P, DIM // 512, 512).transpose(0, 2, 1, 3)
    ).astype(fp16)
    pm = np.zeros((P, P), fp16)
    idx = np.arange(0, P, 2)
    pm[idx + 1, idx] = 1.0
    pm[idx, idx + 1] = 1.0
    ones = np.ones((P, P), fp16)

    inv = ROPE_THETA ** (-np.arange(0, HEAD_DIM, 2, dtype=np.float32) / HEAD_DIM)

    def tables(pos):
        ang = pos[None, :].astype(np.float32) * inv[:, None]  # [64, T]
        C = np.repeat(np.cos(ang), 2, axis=0)
        S = np.repeat(np.sin(ang), 2, axis=0)
        S[0::2] *= -1.0
        return (np.ascontiguousarray(C).astype(np.float16),
                np.ascontiguousarray(S).astype(np.float16))

    ck, sk = tables(np.arange(KV_LEN))

    in_maps = []
    for c in range(N_CORES):
        r0 = c * RQ
        b = r0 // Q_LEN
        qoff = r0 % Q_LEN
        xt_c = np.ascontiguousarray(x[r0:r0 + RQ].T).astype(fp16)
        kt_c = np.ascontiguousarray(xk[b * KV_LEN:(b + 1) * KV_LEN].T).astype(fp16)
        v_c = xv[b * KV_LEN:(b + 1) * KV_LEN].astype(fp16)
        cq = np.empty((N_PASS, P, QP), np.float16)
        sq = np.empty((N_PASS, P, QP), np.float16)
        for p_i in range(N_PASS):
            Cq, Sq = tables(qoff + p_i * QP + np.arange(QP))
            cq[p_i] = Cq
            sq[p_i] = Sq
        in_maps.append({
            "xt": xt_c, "wq": wq_blk, "wo": wo_16, "kt": kt_c, "v": v_c,
            "cq": cq, "sq": sq, "ck": ck, "sk": sk, "pm": pm, "on": ones,
        })
    return in_maps


def run_sharded(inputs, trace=False, trace_kwargs=None):
    """Build/compile (cached), run on cores 0-7, return (full_out, results)."""
    nc = _build()
    in_maps = _host_prep(inputs["x"], inputs["xk"], inputs["xv"],
                         inputs["Wq"], inputs["Wo"])
    kw = {}
    if trace:
        kw["trace"] = True
        if trace_kwargs:
            kw["trace_kwargs"] = trace_kwargs
    res = run_bass_kernel_spmd(nc, in_maps, core_ids=list(range(N_CORES)), **kw)
    full = np.concatenate([res.results[c]["out"] for c in range(N_CORES)],
                          axis=0)
    return full, res


def kernel(**inputs):
    ns = inputs.get("num_seqs", NUM_SEQS)
    assert int(ns) == NUM_SEQS, f"kernel hardcoded for num_seqs={NUM_SEQS}"
    full, _ = run_sharded(inputs, trace=False)
    return full


# revision 16
# speedup vs baseline: 1.8132x; 1.0122x over previous
"""Trainium2 Bass kernel for varlen GQA cross-attention (4 seqs x 2048 q, 512 kv).

Strategy: data-parallel over query rows. Each of the 8 cores owns 1024 query
rows (half of one sequence) and the full 512-slot KV of that sequence.
No collectives needed.

Per-core dataflow (layouts chosen so no on-device transposes are needed):
  xT [4096,1024] (host pre-transposed)  -> Q^T = Wq.T-chunks x xT   [hd, q]
  RoPE on Q^T / K^T via pair-swap permutation matmul + cos/sin tables
  S^T[k,q] = (K^T chunk).T @ Q^T        (contraction over head_dim)
  expS = exp(S^T * scale)  (ScalarE, PSUM->SBUF)
  denom[1,q] = ones.T @ expS            (partition-dim reduction by matmul)
  O^T[hd,q] = V-chunk.T @ expS          (PSUM accum over k chunks)
  O^T *= broadcast(1/denom)             (broadcast via K=1 ones matmul)
  Y[q,n] = O^T-chunks.T @ Wo-chunks     (accumulate over all 32 heads)

All matmul operands are fp16 (1 cycle/row on the PE, fast weight load that
overlaps with matmuls); every accumulation is fp32 in PSUM, and softmax
intermediates stay fp32 on the vector/scalar engines.

The per-head attention chain (PE -> ACT -> PE -> DVE -> PE) is software
pipelined one head deep so the PE always has the next head's 32 independent
Q-projection matmuls to execute while a head's cross-engine chain resolves.
"""

import sys

if "/opt/trn_rl_repo" not in sys.path:
    sys.path.insert(0, "/opt/trn_rl_repo")

import numpy as np
import ml_dtypes
from contextlib import ExitStack

import concourse.bass as bass
import concourse.tile as tile
import concourse.mybir as mybir
from concourse import bacc
from concourse.bass_utils import run_bass_kernel_spmd

# Problem constants (hardcoded per harness contract)
DIM = 4096
N_HEADS = 32
HEAD_DIM = 128
N_KV_HEADS = 8
REPEATS = N_HEADS // N_KV_HEADS
SCALE = HEAD_DIM ** -0.5
ROPE_THETA = 10000.0
NUM_SEQS = 4
Q_LEN = 2048
KV_LEN = 512
N_CORES = 8
RQ = (NUM_SEQS * Q_LEN) // N_CORES   # 1024 query rows per core
QP = 512                              # q rows per pass
N_PASS = RQ // QP                     # 2
P = 128
KC = DIM // P                         # 32 contraction chunks
NKT = KV_LEN // P                     # 4 kv chunks

f32 = mybir.dt.float32
f16 = mybir.dt.float16
Copy = mybir.ActivationFunctionType.Copy
Exp = mybir.ActivationFunctionType.Exp

_BUILT = None


def _build():
    """Build + compile the per-core Bass program (same NEFF on all 8 cores)."""
    global _BUILT
    if _BUILT is not None:
        return _BUILT

    nc = bacc.Bacc("TRN2", target_bir_lowering=False, debug=False,
                   num_devices=N_CORES)
    xt = nc.dram_tensor("xt", [DIM, RQ], f16, kind="ExternalInput").ap()
    wq = nc.dram_tensor("wq", [N_HEADS, P, KC, P], f16,
                        kind="ExternalInput").ap()
    wo = nc.dram_tensor("wo", [N_HEADS, DIM // 512, P, 512], f16,
                    kind="ExternalInput").ap()
    ktd = nc.dram_tensor("kt", [N_KV_HEADS * P, KV_LEN], f16,
                         kind="ExternalInput").ap()
    vd = nc.dram_tensor("v", [KV_LEN, N_KV_HEADS * P], f16,
                        kind="ExternalInput").ap()
    cqd = nc.dram_tensor("cq", [N_PASS, P, QP], f16, kind="ExternalInput").ap()
    sqd = nc.dram_tensor("sq", [N_PASS, P, QP], f16, kind="ExternalInput").ap()
    ckd = nc.dram_tensor("ck", [P, KV_LEN], f16, kind="ExternalInput").ap()
    skd = nc.dram_tensor("sk", [P, KV_LEN], f16, kind="ExternalInput").ap()
    pmd = nc.dram_tensor("pm", [P, P], f16, kind="ExternalInput").ap()
    ond = nc.dram_tensor("on", [P, P], f16, kind="ExternalInput").ap()
    out = nc.dram_tensor("out", [RQ, DIM], f32, kind="ExternalOutput").ap()

    with tile.TileContext(nc) as tc:
        with ExitStack() as ctx:
            singles = ctx.enter_context(tc.tile_pool(name="singles", bufs=1))
            big = ctx.enter_context(tc.tile_pool(name="big", bufs=2))
            wqp = ctx.enter_context(tc.tile_pool(name="wqp", bufs=3))
            wop = ctx.enter_context(tc.tile_pool(name="wop", bufs=5))
            wk = ctx.enter_context(tc.tile_pool(name="wk", bufs=2))
            esp = ctx.enter_context(tc.tile_pool(name="esp", bufs=8))
            psA = ctx.enter_context(tc.tile_pool(name="psA", bufs=4, space="PSUM"))
            ph12 = ExitStack()
            psB = ph12.enter_context(tc.tile_pool(name="psB", bufs=2, space="PSUM"))
            psC = ph12.enter_context(tc.tile_pool(name="psC", bufs=2, space="PSUM"))

            # Prefetch the first two Wq head blocks ahead of the bulk DMAs
            wq_tiles = {}
            for h0 in range(2):
                wq_t = wqp.tile([P, KC, P], f16, tag="wq", name=f"wq_pre{h0}")
                nc.sync.dma_start(wq_t, wq[h0])
                wq_tiles[h0] = wq_t

            # Resident tensors
            kt_sb = singles.tile([P, N_KV_HEADS, KV_LEN], f16)  # K^T post-rope
            v_sb = singles.tile([P, NKT, N_KV_HEADS, P], f16)
            pm_sb = singles.tile([P, P], f16)
            on_sb = singles.tile([P, P], f16)
            ck_sb = singles.tile([P, KV_LEN], f16)
            sk_sb = singles.tile([P, KV_LEN], f16)
            nc.sync.dma_start(
                v_sb, vd.rearrange("(kc kin) (g hd) -> kin kc g hd", kin=P, hd=P))
            nc.sync.dma_start(pm_sb, pmd)
            nc.sync.dma_start(on_sb, ond)
            nc.sync.dma_start(ck_sb, ckd)
            nc.sync.dma_start(sk_sb, skd)

            # RoPE on K^T: kt_sb(f16) = ktf*C + swap(ktf)*S
            for g in range(N_KV_HEADS):
                ktf = wk.tile([P, KV_LEN], f16, tag="ktf")
                nc.sync.dma_start(ktf, ktd[g * P:(g + 1) * P, :])
                ksw = psB.tile([P, KV_LEN], f32, tag="st")
                nc.tensor.matmul(ksw, pm_sb, ktf, start=True, stop=True)
                kt1 = wk.tile([P, KV_LEN], f32, tag="vtmp")
                nc.vector.tensor_mul(kt1, ksw, sk_sb)
                kt2 = wk.tile([P, KV_LEN], f32, tag="y")
                nc.vector.tensor_mul(kt2, ktf, ck_sb)
                nc.vector.tensor_add(kt_sb[:, g], kt2, kt1)

            # Both q-passes resident; units are (head, pass) so each Wq
            # head block is loaded once and serves both passes.
            xts, ots, cqs, sqs = [], [], [], []
            for ps in range(N_PASS):
                xts.append(big.tile([P, KC, QP], f16, tag="xt",
                                    name=f"xt_{ps}"))
                ots.append(big.tile([P, N_HEADS, QP], f16, tag="ot",
                                    name=f"ot_{ps}"))
                cqs.append(big.tile([P, QP], f16, tag="cq", name=f"cq_{ps}"))
                sqs.append(big.tile([P, QP], f16, tag="sq", name=f"sq_{ps}"))

            def emit_pass_loads(ps):
                for kg in range(8):
                    nc.sync.dma_start(
                        xts[ps][:, kg * 4:(kg + 1) * 4, :],
                        xt[kg * 4 * P:(kg + 1) * 4 * P,
                           ps * QP:(ps + 1) * QP].rearrange(
                            "(kc kin) q -> kin kc q", kin=P))
                nc.sync.dma_start(cqs[ps], cqd[ps])
                nc.sync.dma_start(sqs[ps], sqd[ps])

            emit_pass_loads(0)

            NU = N_HEADS * N_PASS
            st_rope = {}
            st_attn = {}

            def unit_hp(u):
                block, r = u // 4, u % 4
                return block * 2 + (r % 2), r // 2

            def emit_qt(u):
                h, ps = unit_hp(u)
                qtp = psA.tile([P, QP], f32, tag="mmout", name=f"qtp_{u}")
                if ps == 0:
                    if h in wq_tiles:
                        wq_t = wq_tiles[h]
                    else:
                        wq_t = wqp.tile([P, KC, P], f16, tag="wq")
                        nc.sync.dma_start(wq_t, wq[h])
                        wq_tiles[h] = wq_t
                else:
                    wq_t = wq_tiles.pop(h)
                for k in range(KC):
                    nc.tensor.matmul(qtp, wq_t[:, k], xts[ps][:, k],
                                     start=(k == 0), stop=(k == KC - 1))
                qt_raw = wk.tile([P, QP], f16, tag="qt_raw",
                                 name=f"qt_raw_{u}")
                nc.scalar.activation(qt_raw, qtp, Copy)
                st_rope[u] = qt_raw

            def emit_rope(u):
                h, ps = unit_hp(u)
                qt_raw = st_rope.pop(u)
                qsw = psA.tile([P, QP], f32, tag="mmout", name=f"qsw_{u}")
                nc.tensor.matmul(qsw, pm_sb, qt_raw, start=True, stop=True)
                t1 = wk.tile([P, QP], f32, tag="vtmp", name=f"t1_{u}")
                nc.vector.tensor_mul(t1, qsw, sqs[ps])
                qt_r16 = wk.tile([P, QP], f16, tag="qt_rope",
                                 name=f"qt_rope_{u}")
                nc.vector.tensor_mul(qt_r16, qt_raw, cqs[ps])
                nc.vector.tensor_add(qt_r16, qt_r16, t1)
                st_rope[u] = qt_r16

            def emit_scores(u):
                h, _ = unit_hp(u)
                qt_r16 = st_rope.pop(u)
                g = h // REPEATS
                ess = []
                for kt_i in range(NKT):
                    stp = psB.tile([P, QP], f32, tag="st",
                                   name=f"stp_{u}_{kt_i}")
                    nc.tensor.matmul(
                        stp, kt_sb[:, g, kt_i * P:(kt_i + 1) * P],
                        qt_r16, start=True, stop=True)
                    es = esp.tile([P, QP], f16, tag="es",
                                  name=f"es_{u}_{kt_i}")
                    nc.scalar.activation(es, stp, Exp, scale=SCALE)
                    ess.append(es)
                st_attn[u] = ess

            def emit_dnav(u):
                h, _ = unit_hp(u)
                ess = st_attn.pop(u)
                g = h // REPEATS
                # Full ones block: every psum partition gets the denominator
                # (same PE cost; no broadcast needed afterwards).
                dnp = psC.tile([P, QP], f32, tag="aux", name=f"dnp_{u}")
                otp = psA.tile([P, QP], f32, tag="mmout", name=f"otp_{u}")
                for kt_i, es in enumerate(ess):
                    nc.tensor.matmul(dnp, on_sb, es,
                                     start=(kt_i == 0),
                                     stop=(kt_i == NKT - 1))
                    nc.tensor.matmul(otp, v_sb[:, kt_i, g], es,
                                     start=(kt_i == 0),
                                     stop=(kt_i == NKT - 1))
                st_attn[u] = (dnp, otp)

            def emit_norm(u):
                h, ps = unit_hp(u)
                dnp, otp = st_attn.pop(u)
                rc = wk.tile([P, QP], f16, tag="rc", name=f"rc_{u}")
                with nc.allow_low_precision(reason="softmax reciprocal"):
                    nc.vector.reciprocal(rc, dnp)
                nc.vector.tensor_mul(ots[ps][:, h], otp, rc)

            # 3-deep software pipeline over (head, pass) units
            for u in range(NU):
                emit_qt(u)
                if u == 1:
                    emit_pass_loads(1)
                if u > 0:
                    emit_scores(u - 1)
                if u > 1:
                    emit_dnav(u - 2)
                if u > 2:
                    emit_norm(u - 3)
                emit_rope(u)
            emit_scores(NU - 1)
            emit_dnav(NU - 2)
            emit_norm(NU - 3)
            emit_dnav(NU - 1)
            emit_norm(NU - 2)
            emit_norm(NU - 1)

            # ---- Phase 3: Y = O @ Wo, both passes share each Wo block ----
            ph12.close()
            with tc.tile_pool(name="psY", bufs=4, space="PSUM") as psY:
                for n in range(DIM // 512):
                    yps = [psA.tile([P, 512], f32, tag="mmout",
                                    name=f"yA_{n}_{m}")
                           for m in range(QP // P)]
                    yps += [psY.tile([P, 512], f32, tag="y2",
                                     name=f"yB_{n}_{m}")
                            for m in range(QP // P)]
                    for h in range(N_HEADS):
                        wo_t = wop.tile([P, 512], f16, tag="wo")
                        nc.sync.dma_start(wo_t, wo[h, n])
                        for ps in range(N_PASS):
                            for m in range(QP // P):
                                nc.tensor.matmul(
                                    yps[ps * 4 + m],
                                    ots[ps][:, h, m * P:(m + 1) * P],
                                    wo_t,
                                    start=(h == 0),
                                    stop=(h == N_HEADS - 1))
                    for ps in range(N_PASS):
                        for m in range(QP // P):
                            ysb = wk.tile([P, 512], f32, tag="y")
                            nc.vector.tensor_copy(ysb, yps[ps * 4 + m])
                            r0 = ps * QP + m * P
                            nc.sync.dma_start(
                                out[r0:r0 + P, n * 512:(n + 1) * 512], ysb)

    nc.compile()
    _BUILT = nc
    return nc


def _host_prep(x, xk, xv, Wq, Wo):
    """Build the per-core input maps (shard + layout + dtype cast only)."""
    x = np.asarray(x, dtype=np.float32)
    xk = np.asarray(xk, dtype=np.float32)
    xv = np.asarray(xv, dtype=np.float32)
    Wq = np.asarray(Wq, dtype=np.float32)
    Wo = np.asarray(Wo, dtype=np.float32)
    fp16 = np.float16

    # Shared (same on all cores)
    wq_blk = np.ascontiguousarray(
        Wq.reshape(KC, P, N_HEADS, P).transpose(2, 1, 0, 3)).astype(fp16)
    wo_16 = np.ascontiguousarray(
        Wo.reshape(N_HEADS, P, DIM // 512, 512).transpose(0, 2, 1, 3)
    ).astype(fp16)
    pm = np.zeros((P, P), fp16)
    idx = np.arange(0, P, 2)
    pm[idx + 1, idx] = 1.0
    pm[idx, idx + 1] = 1.0
    ones = np.ones((P, P), fp16)

    inv = ROPE_THETA ** (-np.arange(0, HEAD_DIM, 2, dtype=np.float32) / HEAD_DIM)

    def tables(pos):
        ang = pos[None, :].astype(np.float32) * inv[:, None]  # [64, T]
        C = np.repeat(np.cos(ang), 2, axis=0)
        S = np.repeat(np.sin(ang), 2, axis=0)
        S[0::2] *= -1.0
        return (np.ascontiguousarray(C).astype(np.float16),
                np.ascontiguousarray(S).astype(np.float16))

    ck, sk = tables(np.arange(KV_LEN))

    in_maps = []
    for c in range(N_CORES):
        r0 = c * RQ
        b = r0 // Q_LEN
        qoff = r0 % Q_LEN
        xt_c = np.ascontiguousarray(x[r0:r0 + RQ].T).astype(fp16)
        kt_c = np.ascontiguousarray(xk[b * KV_LEN:(b + 1) * KV_LEN].T).astype(fp16)
        v_c = xv[b * KV_LEN:(b + 1) * KV_LEN].astype(fp16)
        cq = np.empty((N_PASS, P, QP), np.float16)
        sq = np.empty((N_PASS, P, QP), np.float16)
        for p_i in range(N_PASS):
            Cq, Sq = tables(qoff + p_i * QP + np.arange(QP))
            cq[p_i] = Cq
            sq[p_i] = Sq
        in_maps.append({
            "xt": xt_c, "wq": wq_blk, "wo": wo_16, "kt": kt_c, "v": v_c,
            "cq": cq, "sq": sq, "ck": ck, "sk": sk, "pm": pm, "on": ones,
        })
    return in_maps


def run_sharded(inputs, trace=False, trace_kwargs=None):
    """Build/compile (cached), run on cores 0-7, return (full_out, results)."""
    nc = _build()
    in_maps = _host_prep(inputs["x"], inputs["xk"], inputs["xv"],
                         inputs["Wq"], inputs["Wo"])
    kw = {}
    if trace:
        kw["trace"] = True
        if trace_kwargs:
            kw["trace_kwargs"] = trace_kwargs
    res = run_bass_kernel_spmd(nc, in_maps, core_ids=list(range(N_CORES)), **kw)
    full = np.concatenate([res.results[c]["out"] for c in range(N_CORES)],
                          axis=0)
    return full, res


def kernel(**inputs):
    ns = inputs.get("num_seqs", NUM_SEQS)
    assert int(ns) == NUM_SEQS, f"kernel hardcoded for num_seqs={NUM_SEQS}"
    full, _ = run_sharded(inputs, trace=False)
    return full


# revision 18
# speedup vs baseline: 1.8901x; 1.0424x over previous
"""Trainium2 Bass kernel for varlen GQA cross-attention (4 seqs x 2048 q, 512 kv).

Strategy: data-parallel over query rows. Each of the 8 cores owns 1024 query
rows (half of one sequence) and the full 512-slot KV of that sequence.
No collectives needed.

Per-core dataflow (layouts chosen so no on-device transposes are needed):
  xT [4096,1024] (host pre-transposed)  -> Q^T = Wq.T-chunks x xT   [hd, q]
  RoPE on Q^T / K^T via pair-swap permutation matmul + cos/sin tables
  S^T[k,q] = (K^T chunk).T @ Q^T        (contraction over head_dim)
  expS = exp(S^T * scale)  (ScalarE, PSUM->SBUF)
  denom[1,q] = ones.T @ expS            (partition-dim reduction by matmul)
  O^T[hd,q] = V-chunk.T @ expS          (PSUM accum over k chunks)
  O^T *= broadcast(1/denom)             (broadcast via K=1 ones matmul)
  Y[q,n] = O^T-chunks.T @ Wo-chunks     (accumulate over all 32 heads)

All matmul operands are fp16 (1 cycle/row on the PE, fast weight load that
overlaps with matmuls); every accumulation is fp32 in PSUM, and softmax
intermediates stay fp32 on the vector/scalar engines.

The per-head attention chain (PE -> ACT -> PE -> DVE -> PE) is software
pipelined one head deep so the PE always has the next head's 32 independent
Q-projection matmuls to execute while a head's cross-engine chain resolves.
"""

import sys

if "/opt/trn_rl_repo" not in sys.path:
    sys.path.insert(0, "/opt/trn_rl_repo")

import numpy as np
import ml_dtypes
from contextlib import ExitStack

import concourse.bass as bass
import concourse.tile as tile
import concourse.mybir as mybir
from concourse import bacc
from concourse.bass_utils import run_bass_kernel_spmd

# Problem constants (hardcoded per harness contract)
DIM = 4096
N_HEADS = 32
HEAD_DIM = 128
N_KV_HEADS = 8
REPEATS = N_HEADS // N_KV_HEADS
SCALE = HEAD_DIM ** -0.5
ROPE_THETA = 10000.0
NUM_SEQS = 4
Q_LEN = 2048
KV_LEN = 512
N_CORES = 8
RQ = (NUM_SEQS * Q_LEN) // N_CORES   # 1024 query rows per core
QP = 512                              # q rows per pass
N_PASS = RQ // QP                     # 2
P = 128
KC = DIM // P                         # 32 contraction chunks
NKT = KV_LEN // P                     # 4 kv chunks

f32 = mybir.dt.float32
f16 = mybir.dt.float16
Copy = mybir.ActivationFunctionType.Copy
Exp = mybir.ActivationFunctionType.Exp

_BUILT = None


def _build():
    """Build + compile the per-core Bass program (same NEFF on all 8 cores)."""
    global _BUILT
    if _BUILT is not None:
        return _BUILT

    nc = bacc.Bacc("TRN2", target_bir_lowering=False, debug=False,
                   num_devices=N_CORES)
    xt = nc.dram_tensor("xt", [DIM, RQ], f16, kind="ExternalInput").ap()
    wq = nc.dram_tensor("wq", [N_HEADS, P, KC, P], f16,
                        kind="ExternalInput").ap()
    wo = nc.dram_tensor("wo", [N_HEADS, DIM // 512, P, 512], f16,
                    kind="ExternalInput").ap()
    ktd = nc.dram_tensor("kt", [N_KV_HEADS * P, KV_LEN], f16,
                         kind="ExternalInput").ap()
    vd = nc.dram_tensor("v", [KV_LEN, N_KV_HEADS * P], f16,
                        kind="ExternalInput").ap()
    cqd = nc.dram_tensor("cq", [N_PASS, P, QP], f16, kind="ExternalInput").ap()
    sqd = nc.dram_tensor("sq", [N_PASS, P, QP], f16, kind="ExternalInput").ap()
    ckd = nc.dram_tensor("ck", [P, KV_LEN], f16, kind="ExternalInput").ap()
    skd = nc.dram_tensor("sk", [P, KV_LEN], f16, kind="ExternalInput").ap()
    pmd = nc.dram_tensor("pm", [P, P], f16, kind="ExternalInput").ap()
    ond = nc.dram_tensor("on", [P, P], f16, kind="ExternalInput").ap()
    out = nc.dram_tensor("out", [RQ, DIM], f32, kind="ExternalOutput").ap()

    with tile.TileContext(nc) as tc:
        with ExitStack() as ctx:
            singles = ctx.enter_context(tc.tile_pool(name="singles", bufs=1))
            big = ctx.enter_context(tc.tile_pool(name="big", bufs=2))
            wqp = ctx.enter_context(tc.tile_pool(name="wqp", bufs=3))
            wop = ctx.enter_context(tc.tile_pool(name="wop", bufs=5))
            wk = ctx.enter_context(tc.tile_pool(name="wk", bufs=2))
            esp = ctx.enter_context(tc.tile_pool(name="esp", bufs=8))
            psA = ctx.enter_context(tc.tile_pool(name="psA", bufs=4, space="PSUM"))
            ph12 = ExitStack()
            psB = ph12.enter_context(tc.tile_pool(name="psB", bufs=2, space="PSUM"))
            psC = ph12.enter_context(tc.tile_pool(name="psC", bufs=2, space="PSUM"))

            # Prefetch the first two Wq head blocks ahead of the bulk DMAs
            wq_tiles = {}
            for h0 in range(2):
                wq_t = wqp.tile([P, KC, P], f16, tag="wq", name=f"wq_pre{h0}")
                nc.sync.dma_start(wq_t, wq[h0])
                wq_tiles[h0] = wq_t

            # Resident tensors
            kt_sb = singles.tile([P, N_KV_HEADS, KV_LEN], f16)  # K^T post-rope
            v_sb = singles.tile([P, NKT, N_KV_HEADS, P], f16)
            pm_sb = singles.tile([P, P], f16)
            on_sb = singles.tile([P, P], f16)
            ck_sb = singles.tile([P, KV_LEN], f16)
            sk_sb = singles.tile([P, KV_LEN], f16)
            nc.sync.dma_start(pm_sb, pmd)
            nc.sync.dma_start(on_sb, ond)
            nc.sync.dma_start(ck_sb, ckd)
            nc.sync.dma_start(sk_sb, skd)

            def emit_krope():
                # RoPE on K^T: kt_sb(f16) = ktf*C + swap(ktf)*S. Emitted after
                # unit 0 so the PE's first work needs only wq[0] + one xT chunk.
                nc.sync.dma_start(
                    v_sb,
                    vd.rearrange("(kc kin) (g hd) -> kin kc g hd", kin=P, hd=P))
                for g in range(N_KV_HEADS):
                    ktf = wk.tile([P, KV_LEN], f16, tag="ktf")
                    nc.sync.dma_start(ktf, ktd[g * P:(g + 1) * P, :])
                    ksw = psB.tile([P, KV_LEN], f32, tag="st")
                    nc.tensor.matmul(ksw, pm_sb, ktf, start=True, stop=True)
                    kt1 = wk.tile([P, KV_LEN], f32, tag="vtmp")
                    nc.vector.tensor_mul(kt1, ksw, sk_sb)
                    kt2 = wk.tile([P, KV_LEN], f32, tag="y", bufs=4)
                    nc.vector.tensor_mul(kt2, ktf, ck_sb)
                    nc.vector.tensor_add(kt_sb[:, g], kt2, kt1)

            # Both q-passes resident; units are (head, pass) so each Wq
            # head block is loaded once and serves both passes.
            xts, ots, cqs, sqs = [], [], [], []
            for ps in range(N_PASS):
                xts.append(big.tile([P, KC, QP], f16, tag="xt",
                                    name=f"xt_{ps}"))
                ots.append(big.tile([P, N_HEADS, QP], f16, tag="ot",
                                    name=f"ot_{ps}"))
                cqs.append(big.tile([P, QP], f16, tag="cq", name=f"cq_{ps}"))
                sqs.append(big.tile([P, QP], f16, tag="sq", name=f"sq_{ps}"))

            def emit_pass_loads(ps):
                for kg in range(8):
                    nc.sync.dma_start(
                        xts[ps][:, kg * 4:(kg + 1) * 4, :],
                        xt[kg * 4 * P:(kg + 1) * 4 * P,
                           ps * QP:(ps + 1) * QP].rearrange(
                            "(kc kin) q -> kin kc q", kin=P))
                nc.sync.dma_start(cqs[ps], cqd[ps])
                nc.sync.dma_start(sqs[ps], sqd[ps])

            emit_pass_loads(0)

            NU = N_HEADS * N_PASS
            st_rope = {}
            st_attn = {}

            def unit_hp(u):
                block, r = u // 4, u % 4
                return block * 2 + (r % 2), r // 2

            def emit_qt(u):
                h, ps = unit_hp(u)
                qtp = psA.tile([P, QP], f32, tag="mmout", name=f"qtp_{u}")
                if ps == 0:
                    if h in wq_tiles:
                        wq_t = wq_tiles[h]
                    else:
                        wq_t = wqp.tile([P, KC, P], f16, tag="wq")
                        nc.sync.dma_start(wq_t, wq[h])
                        wq_tiles[h] = wq_t
                else:
                    wq_t = wq_tiles.pop(h)
                for k in range(KC):
                    nc.tensor.matmul(qtp, wq_t[:, k], xts[ps][:, k],
                                     start=(k == 0), stop=(k == KC - 1))
                qt_raw = wk.tile([P, QP], f16, tag="qt_raw",
                                 name=f"qt_raw_{u}")
                nc.scalar.activation(qt_raw, qtp, Copy)
                st_rope[u] = qt_raw

            def emit_rope(u):
                h, ps = unit_hp(u)
                qt_raw = st_rope.pop(u)
                qsw = psA.tile([P, QP], f32, tag="mmout", name=f"qsw_{u}")
                nc.tensor.matmul(qsw, pm_sb, qt_raw, start=True, stop=True)
                t1 = wk.tile([P, QP], f32, tag="vtmp", name=f"t1_{u}")
                nc.vector.tensor_mul(t1, qsw, sqs[ps])
                qt_r16 = wk.tile([P, QP], f16, tag="qt_rope",
                                 name=f"qt_rope_{u}")
                nc.vector.tensor_mul(qt_r16, qt_raw, cqs[ps])
                nc.vector.tensor_add(qt_r16, qt_r16, t1)
                st_rope[u] = qt_r16

            def emit_scores(u):
                h, _ = unit_hp(u)
                qt_r16 = st_rope.pop(u)
                g = h // REPEATS
                ess = []
                for kt_i in range(NKT):
                    stp = psB.tile([P, QP], f32, tag="st",
                                   name=f"stp_{u}_{kt_i}")
                    nc.tensor.matmul(
                        stp, kt_sb[:, g, kt_i * P:(kt_i + 1) * P],
                        qt_r16, start=True, stop=True)
                    es = esp.tile([P, QP], f16, tag="es",
                                  name=f"es_{u}_{kt_i}")
                    nc.scalar.activation(es, stp, Exp, scale=SCALE)
                    ess.append(es)
                st_attn[u] = ess

            def emit_dnav(u):
                h, _ = unit_hp(u)
                ess = st_attn.pop(u)
                g = h // REPEATS
                # Full ones block: every psum partition gets the denominator
                # (same PE cost; no broadcast needed afterwards).
                dnp = psC.tile([P, QP], f32, tag="aux", name=f"dnp_{u}")
                otp = psA.tile([P, QP], f32, tag="mmout", name=f"otp_{u}")
                for kt_i, es in enumerate(ess):
                    nc.tensor.matmul(dnp, on_sb, es,
                                     start=(kt_i == 0),
                                     stop=(kt_i == NKT - 1))
                    nc.tensor.matmul(otp, v_sb[:, kt_i, g], es,
                                     start=(kt_i == 0),
                                     stop=(kt_i == NKT - 1))
                st_attn[u] = (dnp, otp)

            def emit_norm(u):
                h, ps = unit_hp(u)
                dnp, otp = st_attn.pop(u)
                rc = wk.tile([P, QP], f16, tag="rc", name=f"rc_{u}")
                with nc.allow_low_precision(reason="softmax reciprocal"):
                    nc.vector.reciprocal(rc, dnp)
                nc.vector.tensor_mul(ots[ps][:, h], otp, rc)

            # 3-deep software pipeline over (head, pass) units
            for u in range(NU):
                emit_qt(u)
                if u == 1:
                    emit_krope()
                    emit_pass_loads(1)
                if u > 0:
                    emit_scores(u - 1)
                if u > 1:
                    emit_dnav(u - 2)
                if u > 2:
                    emit_norm(u - 3)
                emit_rope(u)
            emit_scores(NU - 1)
            emit_dnav(NU - 2)
            emit_norm(NU - 3)
            emit_dnav(NU - 1)
            emit_norm(NU - 2)
            emit_norm(NU - 1)

            # ---- Phase 3: Y = O @ Wo, both passes share each Wo block ----
            ph12.close()
            with tc.tile_pool(name="psY", bufs=4, space="PSUM") as psY:
                for n in range(DIM // 512):
                    yps = [psA.tile([P, 512], f32, tag="mmout",
                                    name=f"yA_{n}_{m}")
                           for m in range(QP // P)]
                    yps += [psY.tile([P, 512], f32, tag="y2",
                                     name=f"yB_{n}_{m}")
                            for m in range(QP // P)]
                    for h in range(N_HEADS):
                        wo_t = wop.tile([P, 512], f16, tag="wo")
                        nc.sync.dma_start(wo_t, wo[h, n])
                        for ps in range(N_PASS):
                            for m in range(QP // P):
                                nc.tensor.matmul(
                                    yps[ps * 4 + m],
                                    ots[ps][:, h, m * P:(m + 1) * P],
                                    wo_t,
                                    start=(h == 0),
                                    stop=(h == N_HEADS - 1))
                    for ps in range(N_PASS):
                        for m in range(QP // P):
                            ysb = wk.tile([P, 512], f32, tag="y", bufs=4)
                            nc.vector.tensor_copy(ysb, yps[ps * 4 + m])
                            r0 = ps * QP + m * P
                            nc.sync.dma_start(
                                out[r0:r0 + P, n * 512:(n + 1) * 512], ysb)

    nc.compile()
    _BUILT = nc
    return nc


def _host_prep(x, xk, xv, Wq, Wo):
    """Build the per-core input maps (shard + layout + dtype cast only)."""
    x = np.asarray(x, dtype=np.float32)
    xk = np.asarray(xk, dtype=np.float32)
    xv = np.asarray(xv, dtype=np.float32)
    Wq = np.asarray(Wq, dtype=np.float32)
    Wo = np.asarray(Wo, dtype=np.float32)
    fp16 = np.float16

    # Shared (same on all cores)
    wq_blk = np.ascontiguousarray(
        Wq.reshape(KC, P, N_HEADS, P).transpose(2, 1, 0, 3)).astype(fp16)
    wo_16 = np.ascontiguousarray(
        Wo.reshape(N_HEADS, P, DIM // 512, 512).transpose(0, 2, 1, 3)
    ).astype(fp16)
    pm = np.zeros((P, P), fp16)
    idx = np.arange(0, P, 2)
    pm[idx + 1, idx] = 1.0
    pm[idx, idx + 1] = 1.0
    ones = np.ones((P, P), fp16)

    inv = ROPE_THETA ** (-np.arange(0, HEAD_DIM, 2, dtype=np.float32) / HEAD_DIM)

    def tables(pos):
        ang = pos[None, :].astype(np.float32) * inv[:, None]  # [64, T]
        C = np.repeat(np.cos(ang), 2, axis=0)
        S = np.repeat(np.sin(ang), 2, axis=0)
        S[0::2] *= -1.0
        return (np.ascontiguousarray(C).astype(np.float16),
                np.ascontiguousarray(S).astype(np.float16))

    ck, sk = tables(np.arange(KV_LEN))

    in_maps = []
    for c in range(N_CORES):
        r0 = c * RQ
        b = r0 // Q_LEN
        qoff = r0 % Q_LEN
        xt_c = np.ascontiguousarray(x[r0:r0 + RQ].T).astype(fp16)
        kt_c = np.ascontiguousarray(xk[b * KV_LEN:(b + 1) * KV_LEN].T).astype(fp16)
        v_c = xv[b * KV_LEN:(b + 1) * KV_LEN].astype(fp16)
        cq = np.empty((N_PASS, P, QP), np.float16)
        sq = np.empty((N_PASS, P, QP), np.float16)
        for p_i in range(N_PASS):
            Cq, Sq = tables(qoff + p_i * QP + np.arange(QP))
            cq[p_i] = Cq
            sq[p_i] = Sq
        in_maps.append({
            "xt": xt_c, "wq": wq_blk, "wo": wo_16, "kt": kt_c, "v": v_c,
            "cq": cq, "sq": sq, "ck": ck, "sk": sk, "pm": pm, "on": ones,
        })
    return in_maps


def run_sharded(inputs, trace=False, trace_kwargs=None):
    """Build/compile (cached), run on cores 0-7, return (full_out, results)."""
    nc = _build()
    in_maps = _host_prep(inputs["x"], inputs["xk"], inputs["xv"],
                         inputs["Wq"], inputs["Wo"])
    kw = {}
    if trace:
        kw["trace"] = True
        if trace_kwargs:
            kw["trace_kwargs"] = trace_kwargs
    res = run_bass_kernel_spmd(nc, in_maps, core_ids=list(range(N_CORES)), **kw)
    full = np.concatenate([res.results[c]["out"] for c in range(N_CORES)],
                          axis=0)
    return full, res


def kernel(**inputs):
    ns = inputs.get("num_seqs", NUM_SEQS)
    assert int(ns) == NUM_SEQS, f"kernel hardcoded for num_seqs={NUM_SEQS}"
    full, _ = run_sharded(inputs, trace=False)
    return full


# revision 20
# speedup vs baseline: 1.8973x; 1.0038x over previous
"""Trainium2 Bass kernel for varlen GQA cross-attention (4 seqs x 2048 q, 512 kv).

Strategy: data-parallel over query rows. Each of the 8 cores owns 1024 query
rows (half of one sequence) and the full 512-slot KV of that sequence.
No collectives needed.

Per-core dataflow (layouts chosen so no on-device transposes are needed):
  xT [4096,1024] (host pre-transposed)  -> Q^T = Wq.T-chunks x xT   [hd, q]
  RoPE on Q^T / K^T via pair-swap permutation matmul + cos/sin tables
  S^T[k,q] = (K^T chunk).T @ Q^T        (contraction over head_dim)
  expS = exp(S^T * scale)  (ScalarE, PSUM->SBUF)
  denom[1,q] = ones.T @ expS            (partition-dim reduction by matmul)
  O^T[hd,q] = V-chunk.T @ expS          (PSUM accum over k chunks)
  O^T *= broadcast(1/denom)             (broadcast via K=1 ones matmul)
  Y[q,n] = O^T-chunks.T @ Wo-chunks     (accumulate over all 32 heads)

All matmul operands are fp16 (1 cycle/row on the PE, fast weight load that
overlaps with matmuls); every accumulation is fp32 in PSUM, and softmax
intermediates stay fp32 on the vector/scalar engines.

The per-head attention chain (PE -> ACT -> PE -> DVE -> PE) is software
pipelined one head deep so the PE always has the next head's 32 independent
Q-projection matmuls to execute while a head's cross-engine chain resolves.
"""

import sys

if "/opt/trn_rl_repo" not in sys.path:
    sys.path.insert(0, "/opt/trn_rl_repo")

import numpy as np
import ml_dtypes
from contextlib import ExitStack

import concourse.bass as bass
import concourse.tile as tile
import concourse.mybir as mybir
from concourse import bacc
from concourse.bass_utils import run_bass_kernel_spmd

# Problem constants (hardcoded per harness contract)
DIM = 4096
N_HEADS = 32
HEAD_DIM = 128
N_KV_HEADS = 8
REPEATS = N_HEADS // N_KV_HEADS
SCALE = HEAD_DIM ** -0.5
ROPE_THETA = 10000.0
NUM_SEQS = 4
Q_LEN = 2048
KV_LEN = 512
N_CORES = 8
RQ = (NUM_SEQS * Q_LEN) // N_CORES   # 1024 query rows per core
QP = 512                              # q rows per pass
N_PASS = RQ // QP                     # 2
P = 128
KC = DIM // P                         # 32 contraction chunks
NKT = KV_LEN // P                     # 4 kv chunks

f32 = mybir.dt.float32
f16 = mybir.dt.float16
Copy = mybir.ActivationFunctionType.Copy
Exp = mybir.ActivationFunctionType.Exp

_BUILT = None


def _build():
    """Build + compile the per-core Bass program (same NEFF on all 8 cores)."""
    global _BUILT
    if _BUILT is not None:
        return _BUILT

    nc = bacc.Bacc("TRN2", target_bir_lowering=False, debug=False,
                   num_devices=N_CORES)
    xt = nc.dram_tensor("xt", [DIM, RQ], f16, kind="ExternalInput").ap()
    wq = nc.dram_tensor("wq", [N_HEADS, P, KC, P], f16,
                        kind="ExternalInput").ap()
    wo = nc.dram_tensor("wo", [N_HEADS, DIM // 512, P, 512], f16,
                    kind="ExternalInput").ap()
    ktd = nc.dram_tensor("kt", [N_KV_HEADS * P, KV_LEN], f16,
                         kind="ExternalInput").ap()
    ktw = nc.dram_tensor("ktw", [N_KV_HEADS * P, KV_LEN], f16,
                         kind="ExternalInput").ap()
    vd = nc.dram_tensor("v", [KV_LEN, N_KV_HEADS * P], f16,
                        kind="ExternalInput").ap()
    cqd = nc.dram_tensor("cq", [N_PASS, P, QP], f16, kind="ExternalInput").ap()
    sqd = nc.dram_tensor("sq", [N_PASS, P, QP], f16, kind="ExternalInput").ap()
    ckd = nc.dram_tensor("ck", [P, KV_LEN], f16, kind="ExternalInput").ap()
    skd = nc.dram_tensor("sk", [P, KV_LEN], f16, kind="ExternalInput").ap()
    pmd = nc.dram_tensor("pm", [P, P], f16, kind="ExternalInput").ap()
    ond = nc.dram_tensor("on", [P, P], f16, kind="ExternalInput").ap()
    out = nc.dram_tensor("out", [RQ, DIM], f32, kind="ExternalOutput").ap()

    with tile.TileContext(nc) as tc:
        with ExitStack() as ctx:
            singles = ctx.enter_context(tc.tile_pool(name="singles", bufs=1))
            big = ctx.enter_context(tc.tile_pool(name="big", bufs=2))
            wqp = ctx.enter_context(tc.tile_pool(name="wqp", bufs=3))
            wop = ctx.enter_context(tc.tile_pool(name="wop", bufs=4))
            wk = ctx.enter_context(tc.tile_pool(name="wk", bufs=2))
            esp = ctx.enter_context(tc.tile_pool(name="esp", bufs=7))
            psA = ctx.enter_context(tc.tile_pool(name="psA", bufs=4, space="PSUM"))
            ph12 = ExitStack()
            psB = ph12.enter_context(tc.tile_pool(name="psB", bufs=2, space="PSUM"))
            psC = ph12.enter_context(tc.tile_pool(name="psC", bufs=2, space="PSUM"))

            # Prefetch the first Wq head block ahead of the bulk DMAs
            wq_tiles = {}
            wq_t0 = wqp.tile([P, KC, P], f16, tag="wq", name="wq_pre0")
            nc.sync.dma_start(wq_t0, wq[0])
            wq_tiles[0] = wq_t0

            # Resident tensors
            kt_sb = singles.tile([P, N_KV_HEADS, KV_LEN], f16)  # K^T post-rope
            v_sb = singles.tile([P, NKT, N_KV_HEADS, P], f16)
            on_sb = singles.tile([P, P], f16)
            ck_sb = singles.tile([P, KV_LEN], f16)
            sk_sb = singles.tile([P, KV_LEN], f16)
            nc.sync.dma_start(on_sb, ond)
            nc.sync.dma_start(ck_sb, ckd)
            nc.sync.dma_start(sk_sb, skd)

            def emit_krope():
                # RoPE on K^T: kt_sb(f16) = ktf*C + swap(ktf)*S. Emitted after
                # unit 0 so the PE's first work needs only wq[0] + one xT chunk.
                nc.sync.dma_start(
                    v_sb,
                    vd.rearrange("(kc kin) (g hd) -> kin kc g hd", kin=P, hd=P))
                for g in range(N_KV_HEADS):
                    ktf = wk.tile([P, KV_LEN], f16, tag="ktf")
                    nc.sync.dma_start(ktf, ktd[g * P:(g + 1) * P, :])
                    ksw = wk.tile([P, KV_LEN], f16, tag="ktf")
                    nc.sync.dma_start(ksw, ktw[g * P:(g + 1) * P, :])
                    kt1 = wk.tile([P, KV_LEN], f32, tag="vtmp")
                    nc.vector.tensor_mul(kt1, ksw, sk_sb)
                    kt2 = wk.tile([P, KV_LEN], f32, tag="y", bufs=4)
                    nc.vector.tensor_mul(kt2, ktf, ck_sb)
                    nc.vector.tensor_add(kt_sb[:, g], kt2, kt1)

            # Both q-passes resident; units are (head, pass) so each Wq
            # head block is loaded once and serves both passes.
            xts, ots, cqs, sqs = [], [], [], []
            for ps in range(N_PASS):
                xts.append(big.tile([P, KC, QP], f16, tag="xt",
                                    name=f"xt_{ps}"))
                ots.append(big.tile([P, N_HEADS, QP], f16, tag="ot",
                                    name=f"ot_{ps}"))
                cqs.append(big.tile([P, QP], f16, tag="cq", name=f"cq_{ps}"))
                sqs.append(big.tile([P, QP], f16, tag="sq", name=f"sq_{ps}"))

            def emit_pass_loads(ps):
                for kg in range(8):
                    nc.sync.dma_start(
                        xts[ps][:, kg * 4:(kg + 1) * 4, :],
                        xt[kg * 4 * P:(kg + 1) * 4 * P,
                           ps * QP:(ps + 1) * QP].rearrange(
                            "(kc kin) q -> kin kc q", kin=P))
                nc.sync.dma_start(cqs[ps], cqd[ps])
                nc.sync.dma_start(sqs[ps], sqd[ps])

            emit_pass_loads(0)

            NU = N_HEADS * N_PASS
            st_rope = {}
            st_attn = {}

            def unit_hp(u):
                block, r = u // 4, u % 4
                return block * 2 + (r % 2), r // 2

            def emit_qt(u):
                h, ps = unit_hp(u)
                qtp = psA.tile([P, QP], f32, tag="mmout", name=f"qtp_{u}")
                if ps == 0:
                    if h in wq_tiles:
                        wq_t = wq_tiles[h]
                    else:
                        wq_t = wqp.tile([P, KC, P], f16, tag="wq")
                        nc.sync.dma_start(wq_t, wq[h])
                        wq_tiles[h] = wq_t
                else:
                    wq_t = wq_tiles.pop(h)
                for k in range(KC):
                    nc.tensor.matmul(qtp, wq_t[:, k], xts[ps][:, k],
                                     start=(k == 0), stop=(k == KC - 1))
                qt_raw = wk.tile([P, QP], f16, tag="qt_raw",
                                 name=f"qt_raw_{u}")
                nc.scalar.activation(qt_raw, qtp, Copy)
                st_rope[u] = qt_raw

            def emit_rope(u):
                h, ps = unit_hp(u)
                qt_raw = st_rope.pop(u)
                qsw = wk.tile([P, QP], f16, tag="qsw", name=f"qsw_{u}")
                nc.sync.dma_start(qsw[0::2, :], qt_raw[1::2, :])
                nc.sync.dma_start(qsw[1::2, :], qt_raw[0::2, :])
                t1 = wk.tile([P, QP], f32, tag="vtmp", name=f"t1_{u}")
                nc.vector.tensor_mul(t1, qsw, sqs[ps])
                qt_r16 = wk.tile([P, QP], f16, tag="qt_rope",
                                 name=f"qt_rope_{u}")
                nc.vector.tensor_mul(qt_r16, qt_raw, cqs[ps])
                nc.vector.tensor_add(qt_r16, qt_r16, t1)
                st_rope[u] = qt_r16

            def emit_scores(u):
                h, _ = unit_hp(u)
                qt_r16 = st_rope.pop(u)
                g = h // REPEATS
                ess = []
                for kt_i in range(NKT):
                    stp = psB.tile([P, QP], f32, tag="st",
                                   name=f"stp_{u}_{kt_i}")
                    nc.tensor.matmul(
                        stp, kt_sb[:, g, kt_i * P:(kt_i + 1) * P],
                        qt_r16, start=True, stop=True)
                    es = esp.tile([P, QP], f16, tag="es",
                                  name=f"es_{u}_{kt_i}")
                    nc.scalar.activation(es, stp, Exp, scale=SCALE)
                    ess.append(es)
                st_attn[u] = ess

            def emit_dnav(u):
                h, _ = unit_hp(u)
                ess = st_attn.pop(u)
                g = h // REPEATS
                # Full ones block: every psum partition gets the denominator
                # (same PE cost; no broadcast needed afterwards).
                dnp = psC.tile([P, QP], f32, tag="aux", name=f"dnp_{u}")
                otp = psA.tile([P, QP], f32, tag="mmout", name=f"otp_{u}")
                for kt_i, es in enumerate(ess):
                    nc.tensor.matmul(dnp, on_sb, es,
                                     start=(kt_i == 0),
                                     stop=(kt_i == NKT - 1))
                    nc.tensor.matmul(otp, v_sb[:, kt_i, g], es,
                                     start=(kt_i == 0),
                                     stop=(kt_i == NKT - 1))
                st_attn[u] = (dnp, otp)

            def emit_norm(u):
                h, ps = unit_hp(u)
                dnp, otp = st_attn.pop(u)
                rc = wk.tile([P, QP], f16, tag="rc", name=f"rc_{u}")
                with nc.allow_low_precision(reason="softmax reciprocal"):
                    nc.vector.reciprocal(rc, dnp)
                nc.vector.tensor_mul(ots[ps][:, h], otp, rc)

            # 3-deep software pipeline over (head, pass) units
            for u in range(NU):
                emit_qt(u)
                if u == 1:
                    emit_krope()
                    emit_pass_loads(1)
                if u > 0:
                    emit_scores(u - 1)
                if u > 1:
                    emit_dnav(u - 2)
                if u > 2:
                    emit_norm(u - 3)
                emit_rope(u)
            emit_scores(NU - 1)
            emit_dnav(NU - 2)
            emit_norm(NU - 3)
            emit_dnav(NU - 1)
            emit_norm(NU - 2)
            emit_norm(NU - 1)

            # ---- Phase 3: Y = O @ Wo, both passes share each Wo block ----
            ph12.close()
            with tc.tile_pool(name="psY", bufs=4, space="PSUM") as psY:
                for n in range(DIM // 512):
                    yps = [psA.tile([P, 512], f32, tag="mmout",
                                    name=f"yA_{n}_{m}")
                           for m in range(QP // P)]
                    yps += [psY.tile([P, 512], f32, tag="y2",
                                     name=f"yB_{n}_{m}")
                            for m in range(QP // P)]
                    for h in range(N_HEADS):
                        wo_t = wop.tile([P, 512], f16, tag="wo")
                        nc.sync.dma_start(wo_t, wo[h, n])
                        for ps in range(N_PASS):
                            for m in range(QP // P):
                                nc.tensor.matmul(
                                    yps[ps * 4 + m],
                                    ots[ps][:, h, m * P:(m + 1) * P],
                                    wo_t,
                                    start=(h == 0),
                                    stop=(h == N_HEADS - 1))
                    for ps in range(N_PASS):
                        for m in range(QP // P):
                            ysb = wk.tile([P, 512], f32, tag="y", bufs=4)
                            nc.vector.tensor_copy(ysb, yps[ps * 4 + m])
                            r0 = ps * QP + m * P
                            nc.sync.dma_start(
                                out[r0:r0 + P, n * 512:(n + 1) * 512], ysb)

    nc.compile()
    _BUILT = nc
    return nc


def _host_prep(x, xk, xv, Wq, Wo):
    """Build the per-core input maps (shard + layout + dtype cast only)."""
    x = np.asarray(x, dtype=np.float32)
    xk = np.asarray(xk, dtype=np.float32)
    xv = np.asarray(xv, dtype=np.float32)
    Wq = np.asarray(Wq, dtype=np.float32)
    Wo = np.asarray(Wo, dtype=np.float32)
    fp16 = np.float16

    # Shared (same on all cores)
    wq_blk = np.ascontiguousarray(
        Wq.reshape(KC, P, N_HEADS, P).transpose(2, 1, 0, 3)).astype(fp16)
    wo_16 = np.ascontiguousarray(
        Wo.reshape(N_HEADS, P, DIM // 512, 512).transpose(0, 2, 1, 3)
    ).astype(fp16)
    pm = np.zeros((P, P), fp16)
    idx = np.arange(0, P, 2)
    pm[idx + 1, idx] = 1.0
    pm[idx, idx + 1] = 1.0
    ones = np.ones((P, P), fp16)

    inv = ROPE_THETA ** (-np.arange(0, HEAD_DIM, 2, dtype=np.float32) / HEAD_DIM)

    def tables(pos):
        ang = pos[None, :].astype(np.float32) * inv[:, None]  # [64, T]
        C = np.repeat(np.cos(ang), 2, axis=0)
        S = np.repeat(np.sin(ang), 2, axis=0)
        S[0::2] *= -1.0
        return (np.ascontiguousarray(C).astype(np.float16),
                np.ascontiguousarray(S).astype(np.float16))

    ck, sk = tables(np.arange(KV_LEN))

    in_maps = []
    for c in range(N_CORES):
        r0 = c * RQ
        b = r0 // Q_LEN
        qoff = r0 % Q_LEN
        xt_c = np.ascontiguousarray(x[r0:r0 + RQ].T).astype(fp16)
        kt_c = np.ascontiguousarray(xk[b * KV_LEN:(b + 1) * KV_LEN].T).astype(fp16)
        ktw_c = np.ascontiguousarray(
            kt_c.reshape(N_KV_HEADS, HEAD_DIM // 2, 2, KV_LEN)[:, :, ::-1, :]
        ).reshape(N_KV_HEADS * P, KV_LEN)
        v_c = xv[b * KV_LEN:(b + 1) * KV_LEN].astype(fp16)
        cq = np.empty((N_PASS, P, QP), np.float16)
        sq = np.empty((N_PASS, P, QP), np.float16)
        for p_i in range(N_PASS):
            Cq, Sq = tables(qoff + p_i * QP + np.arange(QP))
            cq[p_i] = Cq
            sq[p_i] = Sq
        in_maps.append({
            "xt": xt_c, "wq": wq_blk, "wo": wo_16, "kt": kt_c, "ktw": ktw_c,
            "v": v_c,
            "cq": cq, "sq": sq, "ck": ck, "sk": sk, "pm": pm, "on": ones,
        })
    return in_maps


def run_sharded(inputs, trace=False, trace_kwargs=None):
    """Build/compile (cached), run on cores 0-7, return (full_out, results)."""
    nc = _build()
    in_maps = _host_prep(inputs["x"], inputs["xk"], inputs["xv"],
                         inputs["Wq"], inputs["Wo"])
    kw = {}
    if trace:
        kw["trace"] = True
        if trace_kwargs:
            kw["trace_kwargs"] = trace_kwargs
    res = run_bass_kernel_spmd(nc, in_maps, core_ids=list(range(N_CORES)), **kw)
    full = np.concatenate([res.results[c]["out"] for c in range(N_CORES)],
                          axis=0)
    return full, res


def kernel(**inputs):
    ns = inputs.get("num_seqs", NUM_SEQS)
    assert int(ns) == NUM_SEQS, f"kernel hardcoded for num_seqs={NUM_SEQS}"
    full, _ = run_sharded(inputs, trace=False)
    return full


# revision 21
# speedup vs baseline: 1.8980x; 1.0004x over previous
"""Trainium2 Bass kernel for varlen GQA cross-attention (4 seqs x 2048 q, 512 kv).

Strategy: data-parallel over query rows. Each of the 8 cores owns 1024 query
rows (half of one sequence) and the full 512-slot KV of that sequence.
No collectives needed.

Per-core dataflow (layouts chosen so no on-device transposes are needed):
  xT [4096,1024] (host pre-transposed)  -> Q^T = Wq.T-chunks x xT   [hd, q]
  RoPE on Q^T / K^T via pair-swap permutation matmul + cos/sin tables
  S^T[k,q] = (K^T chunk).T @ Q^T        (contraction over head_dim)
  expS = exp(S^T * scale)  (ScalarE, PSUM->SBUF)
  denom[1,q] = ones.T @ expS            (partition-dim reduction by matmul)
  O^T[hd,q] = V-chunk.T @ expS          (PSUM accum over k chunks)
  O^T *= broadcast(1/denom)             (broadcast via K=1 ones matmul)
  Y[q,n] = O^T-chunks.T @ Wo-chunks     (accumulate over all 32 heads)

All matmul operands are fp16 (1 cycle/row on the PE, fast weight load that
overlaps with matmuls); every accumulation is fp32 in PSUM, and softmax
intermediates stay fp32 on the vector/scalar engines.

The per-head attention chain (PE -> ACT -> PE -> DVE -> PE) is software
pipelined one head deep so the PE always has the next head's 32 independent
Q-projection matmuls to execute while a head's cross-engine chain resolves.
"""

import sys

if "/opt/trn_rl_repo" not in sys.path:
    sys.path.insert(0, "/opt/trn_rl_repo")

import numpy as np
import ml_dtypes
from contextlib import ExitStack

import concourse.bass as bass
import concourse.tile as tile
import concourse.mybir as mybir
from concourse import bacc
from concourse.bass_utils import run_bass_kernel_spmd

# Problem constants (hardcoded per harness contract)
DIM = 4096
N_HEADS = 32
HEAD_DIM = 128
N_KV_HEADS = 8
REPEATS = N_HEADS // N_KV_HEADS
SCALE = HEAD_DIM ** -0.5
ROPE_THETA = 10000.0
NUM_SEQS = 4
Q_LEN = 2048
KV_LEN = 512
N_CORES = 8
RQ = (NUM_SEQS * Q_LEN) // N_CORES   # 1024 query rows per core
QP = 512                              # q rows per pass
N_PASS = RQ // QP                     # 2
P = 128
KC = DIM // P                         # 32 contraction chunks
NKT = KV_LEN // P                     # 4 kv chunks

f32 = mybir.dt.float32
f16 = mybir.dt.float16
Copy = mybir.ActivationFunctionType.Copy
Exp = mybir.ActivationFunctionType.Exp

_BUILT = None


def _build():
    """Build + compile the per-core Bass program (same NEFF on all 8 cores)."""
    global _BUILT
    if _BUILT is not None:
        return _BUILT

    nc = bacc.Bacc("TRN2", target_bir_lowering=False, debug=False,
                   num_devices=N_CORES)
    xt = nc.dram_tensor("xt", [DIM, RQ], f16, kind="ExternalInput").ap()
    wq = nc.dram_tensor("wq", [N_HEADS, P, KC, P], f16,
                        kind="ExternalInput").ap()
    wo = nc.dram_tensor("wo", [N_HEADS, DIM // 512, P, 512], f16,
                    kind="ExternalInput").ap()
    ktd = nc.dram_tensor("kt", [N_KV_HEADS * P, KV_LEN], f16,
                         kind="ExternalInput").ap()
    ktw = nc.dram_tensor("ktw", [N_KV_HEADS * P, KV_LEN], f16,
                         kind="ExternalInput").ap()
    vd = nc.dram_tensor("v", [KV_LEN, N_KV_HEADS * P], f16,
                        kind="ExternalInput").ap()
    cqd = nc.dram_tensor("cq", [N_PASS, P, QP], f16, kind="ExternalInput").ap()
    sqd = nc.dram_tensor("sq", [N_PASS, P, QP], f16, kind="ExternalInput").ap()
    ckd = nc.dram_tensor("ck", [P, KV_LEN], f16, kind="ExternalInput").ap()
    skd = nc.dram_tensor("sk", [P, KV_LEN], f16, kind="ExternalInput").ap()
    pmd = nc.dram_tensor("pm", [P, P], f16, kind="ExternalInput").ap()
    ond = nc.dram_tensor("on", [P, P], f16, kind="ExternalInput").ap()
    out = nc.dram_tensor("out", [RQ, DIM], f32, kind="ExternalOutput").ap()

    with tile.TileContext(nc) as tc:
        with ExitStack() as ctx:
            singles = ctx.enter_context(tc.tile_pool(name="singles", bufs=1))
            big = ctx.enter_context(tc.tile_pool(name="big", bufs=2))
            wqp = ctx.enter_context(tc.tile_pool(name="wqp", bufs=3))
            wop = ctx.enter_context(tc.tile_pool(name="wop", bufs=4))
            wk = ctx.enter_context(tc.tile_pool(name="wk", bufs=2))
            esp = ctx.enter_context(tc.tile_pool(name="esp", bufs=7))
            psA = ctx.enter_context(tc.tile_pool(name="psA", bufs=4, space="PSUM"))
            ph12 = ExitStack()
            psB = ph12.enter_context(tc.tile_pool(name="psB", bufs=2, space="PSUM"))
            psC = ph12.enter_context(tc.tile_pool(name="psC", bufs=2, space="PSUM"))

            # Prefetch the first Wq head block ahead of the bulk DMAs
            wq_tiles = {}
            wq_t0 = wqp.tile([P, KC, P], f16, tag="wq", name="wq_pre0")
            nc.sync.dma_start(wq_t0, wq[0])
            wq_tiles[0] = wq_t0

            # Resident tensors
            kt_sb = singles.tile([P, N_KV_HEADS, KV_LEN], f16)  # K^T post-rope
            v_sb = singles.tile([P, NKT, N_KV_HEADS, P], f16)
            on_sb = singles.tile([P, P], f16)
            ck_sb = singles.tile([P, KV_LEN], f16)
            sk_sb = singles.tile([P, KV_LEN], f16)
            nc.sync.dma_start(on_sb, ond)
            nc.sync.dma_start(ck_sb, ckd)
            nc.sync.dma_start(sk_sb, skd)

            def emit_krope():
                # RoPE on K^T: kt_sb(f16) = ktf*C + swap(ktf)*S. Emitted after
                # unit 0 so the PE's first work needs only wq[0] + one xT chunk.
                nc.sync.dma_start(
                    v_sb,
                    vd.rearrange("(kc kin) (g hd) -> kin kc g hd", kin=P, hd=P))
                for g in range(N_KV_HEADS):
                    ktf = wk.tile([P, KV_LEN], f16, tag="ktf")
                    nc.sync.dma_start(ktf, ktd[g * P:(g + 1) * P, :])
                    ksw = wk.tile([P, KV_LEN], f16, tag="ktf")
                    nc.sync.dma_start(ksw, ktw[g * P:(g + 1) * P, :])
                    kt1 = wk.tile([P, KV_LEN], f32, tag="vtmp")
                    nc.vector.tensor_mul(kt1, ksw, sk_sb)
                    kt2 = wk.tile([P, KV_LEN], f32, tag="y", bufs=4)
                    nc.vector.tensor_mul(kt2, ktf, ck_sb)
                    nc.vector.tensor_add(kt_sb[:, g], kt2, kt1)

            # Both q-passes resident; units are (head, pass) so each Wq
            # head block is loaded once and serves both passes.
            xts, ots, cqs, sqs = [], [], [], []
            for ps in range(N_PASS):
                xts.append(big.tile([P, KC, QP], f16, tag="xt",
                                    name=f"xt_{ps}"))
                ots.append(big.tile([P, N_HEADS, QP], f16, tag="ot",
                                    name=f"ot_{ps}"))
                cqs.append(big.tile([P, QP], f16, tag="cq", name=f"cq_{ps}"))
                sqs.append(big.tile([P, QP], f16, tag="sq", name=f"sq_{ps}"))

            def emit_pass_loads(ps):
                for kg in range(8):
                    nc.sync.dma_start(
                        xts[ps][:, kg * 4:(kg + 1) * 4, :],
                        xt[kg * 4 * P:(kg + 1) * 4 * P,
                           ps * QP:(ps + 1) * QP].rearrange(
                            "(kc kin) q -> kin kc q", kin=P))
                nc.sync.dma_start(cqs[ps], cqd[ps])
                nc.sync.dma_start(sqs[ps], sqd[ps])

            emit_pass_loads(0)

            NU = N_HEADS * N_PASS
            st_rope = {}
            st_attn = {}

            def unit_hp(u):
                block, r = u // 4, u % 4
                return block * 2 + (r % 2), r // 2

            def emit_qt(u):
                h, ps = unit_hp(u)
                qtp = psA.tile([P, QP], f32, tag="mmout", name=f"qtp_{u}")
                if ps == 0:
                    if h in wq_tiles:
                        wq_t = wq_tiles[h]
                    else:
                        wq_t = wqp.tile([P, KC, P], f16, tag="wq")
                        nc.sync.dma_start(wq_t, wq[h])
                        wq_tiles[h] = wq_t
                else:
                    wq_t = wq_tiles.pop(h)
                for k in range(KC):
                    nc.tensor.matmul(qtp, wq_t[:, k], xts[ps][:, k],
                                     start=(k == 0), stop=(k == KC - 1))
                qt_raw = wk.tile([P, QP], f16, tag="qt_raw",
                                 name=f"qt_raw_{u}")
                nc.scalar.activation(qt_raw, qtp, Copy)
                st_rope[u] = qt_raw

            def emit_rope(u):
                h, ps = unit_hp(u)
                qt_raw = st_rope.pop(u)
                qsw = wk.tile([P, QP], f16, tag="qsw", name=f"qsw_{u}")
                nc.gpsimd.dma_start(qsw[0::2, :], qt_raw[1::2, :])
                nc.gpsimd.dma_start(qsw[1::2, :], qt_raw[0::2, :])
                t1 = wk.tile([P, QP], f32, tag="vtmp", name=f"t1_{u}")
                nc.vector.tensor_mul(t1, qsw, sqs[ps])
                qt_r16 = wk.tile([P, QP], f16, tag="qt_rope",
                                 name=f"qt_rope_{u}")
                nc.vector.tensor_mul(qt_r16, qt_raw, cqs[ps])
                nc.vector.tensor_add(qt_r16, qt_r16, t1)
                st_rope[u] = qt_r16

            def emit_scores(u):
                h, _ = unit_hp(u)
                qt_r16 = st_rope.pop(u)
                g = h // REPEATS
                ess = []
                for kt_i in range(NKT):
                    stp = psB.tile([P, QP], f32, tag="st",
                                   name=f"stp_{u}_{kt_i}")
                    nc.tensor.matmul(
                        stp, kt_sb[:, g, kt_i * P:(kt_i + 1) * P],
                        qt_r16, start=True, stop=True)
                    es = esp.tile([P, QP], f16, tag="es",
                                  name=f"es_{u}_{kt_i}")
                    nc.scalar.activation(es, stp, Exp, scale=SCALE)
                    ess.append(es)
                st_attn[u] = ess

            def emit_dnav(u):
                h, _ = unit_hp(u)
                ess = st_attn.pop(u)
                g = h // REPEATS
                # Full ones block: every psum partition gets the denominator
                # (same PE cost; no broadcast needed afterwards).
                dnp = psC.tile([P, QP], f32, tag="aux", name=f"dnp_{u}")
                otp = psA.tile([P, QP], f32, tag="mmout", name=f"otp_{u}")
                for kt_i, es in enumerate(ess):
                    nc.tensor.matmul(dnp, on_sb, es,
                                     start=(kt_i == 0),
                                     stop=(kt_i == NKT - 1))
                    nc.tensor.matmul(otp, v_sb[:, kt_i, g], es,
                                     start=(kt_i == 0),
                                     stop=(kt_i == NKT - 1))
                st_attn[u] = (dnp, otp)

            def emit_norm(u):
                h, ps = unit_hp(u)
                dnp, otp = st_attn.pop(u)
                rc = wk.tile([P, QP], f16, tag="rc", name=f"rc_{u}")
                with nc.allow_low_precision(reason="softmax reciprocal"):
                    nc.vector.reciprocal(rc, dnp)
                nc.vector.tensor_mul(ots[ps][:, h], otp, rc)

            # 3-deep software pipeline over (head, pass) units
            for u in range(NU):
                emit_qt(u)
                if u == 1:
                    emit_krope()
                    emit_pass_loads(1)
                if u > 0:
                    emit_scores(u - 1)
                if u > 1:
                    emit_dnav(u - 2)
                if u > 2:
                    emit_norm(u - 3)
                emit_rope(u)
            emit_scores(NU - 1)
            emit_dnav(NU - 2)
            emit_norm(NU - 3)
            emit_dnav(NU - 1)
            emit_norm(NU - 2)
            emit_norm(NU - 1)

            # ---- Phase 3: Y = O @ Wo, both passes share each Wo block ----
            ph12.close()
            with tc.tile_pool(name="psY", bufs=4, space="PSUM") as psY:
                for n in range(DIM // 512):
                    yps = [psA.tile([P, 512], f32, tag="mmout",
                                    name=f"yA_{n}_{m}")
                           for m in range(QP // P)]
                    yps += [psY.tile([P, 512], f32, tag="y2",
                                     name=f"yB_{n}_{m}")
                            for m in range(QP // P)]
                    for h in range(N_HEADS):
                        wo_t = wop.tile([P, 512], f16, tag="wo")
                        nc.sync.dma_start(wo_t, wo[h, n])
                        for ps in range(N_PASS):
                            for m in range(QP // P):
                                nc.tensor.matmul(
                                    yps[ps * 4 + m],
                                    ots[ps][:, h, m * P:(m + 1) * P],
                                    wo_t,
                                    start=(h == 0),
                                    stop=(h == N_HEADS - 1))
                    for ps in range(N_PASS):
                        for m in range(QP // P):
                            ysb = wk.tile([P, 512], f32, tag="y", bufs=4)
                            nc.vector.tensor_copy(ysb, yps[ps * 4 + m])
                            r0 = ps * QP + m * P
                            nc.sync.dma_start(
                                out[r0:r0 + P, n * 512:(n + 1) * 512], ysb)

    nc.compile()
    _BUILT = nc
    return nc


def _host_prep(x, xk, xv, Wq, Wo):
    """Build the per-core input maps (shard + layout + dtype cast only)."""
    x = np.asarray(x, dtype=np.float32)
    xk = np.asarray(xk, dtype=np.float32)
    xv = np.asarray(xv, dtype=np.float32)
    Wq = np.asarray(Wq, dtype=np.float32)
    Wo = np.asarray(Wo, dtype=np.float32)
    fp16 = np.float16

    # Shared (same on all cores)
    wq_blk = np.ascontiguousarray(
        Wq.reshape(KC, P, N_HEADS, P).transpose(2, 1, 0, 3)).astype(fp16)
    wo_16 = np.ascontiguousarray(
        Wo.reshape(N_HEADS, P, DIM // 512, 512).transpose(0, 2, 1, 3)
    ).astype(fp16)
    pm = np.zeros((P, P), fp16)
    idx = np.arange(0, P, 2)
    pm[idx + 1, idx] = 1.0
    pm[idx, idx + 1] = 1.0
    ones = np.ones((P, P), fp16)

    inv = ROPE_THETA ** (-np.arange(0, HEAD_DIM, 2, dtype=np.float32) / HEAD_DIM)

    def tables(pos):
        ang = pos[None, :].astype(np.float32) * inv[:, None]  # [64, T]
        C = np.repeat(np.cos(ang), 2, axis=0)
        S = np.repeat(np.sin(ang), 2, axis=0)
        S[0::2] *= -1.0
        return (np.ascontiguousarray(C).astype(np.float16),
                np.ascontiguousarray(S).astype(np.float16))

    ck, sk = tables(np.arange(KV_LEN))

    in_maps = []
    for c in range(N_CORES):
        r0 = c * RQ
        b = r0 // Q_LEN
        qoff = r0 % Q_LEN
        xt_c = np.ascontiguousarray(x[r0:r0 + RQ].T).astype(fp16)
        kt_c = np.ascontiguousarray(xk[b * KV_LEN:(b + 1) * KV_LEN].T).astype(fp16)
        ktw_c = np.ascontiguousarray(
            kt_c.reshape(N_KV_HEADS, HEAD_DIM // 2, 2, KV_LEN)[:, :, ::-1, :]
        ).reshape(N_KV_HEADS * P, KV_LEN)
        v_c = xv[b * KV_LEN:(b + 1) * KV_LEN].astype(fp16)
        cq = np.empty((N_PASS, P, QP), np.float16)
        sq = np.empty((N_PASS, P, QP), np.float16)
        for p_i in range(N_PASS):
            Cq, Sq = tables(qoff + p_i * QP + np.arange(QP))
            cq[p_i] = Cq
            sq[p_i] = Sq
        in_maps.append({
            "xt": xt_c, "wq": wq_blk, "wo": wo_16, "kt": kt_c, "ktw": ktw_c,
            "v": v_c,
            "cq": cq, "sq": sq, "ck": ck, "sk": sk, "pm": pm, "on": ones,
        })
    return in_maps


def run_sharded(inputs, trace=False, trace_kwargs=None):
    """Build/compile (cached), run on cores 0-7, return (full_out, results)."""
    nc = _build()
    in_maps = _host_prep(inputs["x"], inputs["xk"], inputs["xv"],
                         inputs["Wq"], inputs["Wo"])
    kw = {}
    if trace:
        kw["trace"] = True
        if trace_kwargs:
            kw["trace_kwargs"] = trace_kwargs
    res = run_bass_kernel_spmd(nc, in_maps, core_ids=list(range(N_CORES)), **kw)
    full = np.concatenate([res.results[c]["out"] for c in range(N_CORES)],
                          axis=0)
    return full, res


def kernel(**inputs):
    ns = inputs.get("num_seqs", NUM_SEQS)
    assert int(ns) == NUM_SEQS, f"kernel hardcoded for num_seqs={NUM_SEQS}"
    full, _ = run_sharded(inputs, trace=False)
    return full
